# revision 65
# baseline (speedup 1.0000x reference)
"""GAT (2-layer GATConv + FF head) on 8 Trainium2 NeuronCores.

Strategy (per sharding hint): nodes + incident edges partitioned by
destination across 8 cores; per-edge softmax/scatter local to the
destination shard via one-hot matmul-scatter into PSUM; small weights
replicated.

v3 layout: both layers are symmetric [sharded dense -> AllGather h-table ->
edge phase]. The dense transform uses an augmented weight [W | v_s | v_d] so
each node row carries [h | a_src | a_dst] in one matmul; [h | a_src] goes to
the gather table (768B-stride rows: SWDGE rows must be 256B multiples, so
260 payload cols pad to 384), a_dst to a local [NT,H] stash. The per-edge
gather then delivers a_src for free -- no per-edge DVE mult+reduce.

Edge phase per tile-group: gather rows, build the one-hot on DVE (packed-
pair views keep every operand 2-byte/packed-last-dim -> DVE 2x mode), get
the transposed one-hot via PE transpose + Act PSUM->SBUF copy, a_dst via a
one-hot matmul, alpha/lrelu on DVE, exp on Act straight into the row's
a_src slot so the scatter is ONE 260-col matmul per chunk (a single PSUM
accumulation group -- interleaved groups mis-accumulate on HW).

The emission is software-pipelined FRONT(i)/BACK(i-1)/EVICT(i-2): engine
queues execute in-order, so without it alpha's psA wait and evict's PSUM
read head-of-line-block the next group's independent work (~60% of the
edge-phase time).

Message path in fp16 (tables, gathered rows, attention weights, matmul
operands); accumulation in fp32 PSUM; per-edge logits in fp32.
"""
import sys
sys.path.insert(0, "/opt/trn_rl_repo")

import numpy as np
from contextlib import ExitStack

import concourse.bass as bass
import concourse.bacc as bacc
import concourse.tile as tile
import concourse.mybir as mybir
from concourse.bass_utils import run_bass_kernel_spmd

dt = mybir.dt
OP = mybir.AluOpType
ACT = mybir.ActivationFunctionType

NCORES = 8
H = 4
NEG_SLOPE = 0.2


# ----------------------------------------------------------------------------
# host-side prep
# ----------------------------------------------------------------------------

def _wrap_idx(idx):
    """Pack an index list into the SWDGE wrapped layout [128, n/16] int16:
    index i -> partition i%16 (replicated to all 8 16-partition groups),
    free offset i//16."""
    n = len(idx)
    assert n % 128 == 0
    out = np.zeros((128, n // 16), np.int16)
    a = np.asarray(idx, np.int16).reshape(n // 16, 16).T  # [16, n/16]
    for r in range(8):
        out[r * 16:(r + 1) * 16, :] = a
    return out


def _pad128(a, fill):
    n = len(a)
    m = ((n + 127) // 128) * 128
    return np.concatenate([a, np.full(m - n, fill, a.dtype)])


class Sched:
    """Static, core-uniform per-tile chunk schedule."""

    def __init__(self, n_lo, n_hi):
        self.n_lo = n_lo          # [NT] chunks for lo-half gathers
        self.n_hi = n_hi          # [NT] chunks for hi-half gathers
        self.ct = [a + b for a, b in zip(n_lo, n_hi)]
        self.base = np.concatenate([[0], np.cumsum(self.ct)]).astype(int)
        self.total = int(self.base[-1])  # total chunks per core


def _prep(x, edge_index, W1, att_src1, att_dst1, b1, W2, att_src2, att_dst2,
          b2, ff1_w, ff1_b, ff2_w, ff2_b):
    N, IN = x.shape
    F = W1.shape[1]               # 256
    C1 = F // H
    C2 = W2.shape[1] // H
    NSH = N // NCORES
    NT = (NSH + 127) // 128
    # lo/hi split is per-shard (local rows [0:NSH_LO) of each core's shard
    # go to the lo table) so each half can be AllGathered separately and
    # overlap with the other half's gathers. 128-aligned; both halves'
    # table sizes stay within int16 gather indexing.
    NSH_LO = ((NSH // 2) // 128) * 128
    NSH_HI = NSH - NSH_LO

    E = edge_index.shape[1]
    ar = np.arange(N, dtype=np.int64)
    src = np.concatenate([edge_index[0], ar])
    dst = np.concatenate([edge_index[1], ar])

    shard = dst // NSH
    dstloc_all = dst - shard * NSH

    # group edges per (core, tile, half)
    per = [[[None, None] for _ in range(NT)] for _ in range(NCORES)]
    for k in range(NCORES):
        m = shard == k
        s_k, dl_k = src[m], dstloc_all[m]
        t_k = dl_k // 128
        for t in range(NT):
            mt = t_k == t
            s_t, dl_t = s_k[mt], dl_k[mt]
            s_shard, s_loc = s_t // NSH, s_t % NSH
            lo = s_loc < NSH_LO
            per[k][t][0] = (s_shard[lo] * NSH_LO + s_loc[lo], dl_t[lo])
            per[k][t][1] = (s_shard[~lo] * NSH_HI + (s_loc[~lo] - NSH_LO),
                            dl_t[~lo])

    n_lo = [max((len(per[k][t][0][0]) + 127) // 128 for k in range(NCORES))
            for t in range(NT)]
    n_hi = [max((len(per[k][t][1][0]) + 127) // 128 for k in range(NCORES))
            for t in range(NT)]
    sched = Sched(n_lo, n_hi)

    # Per-group chunk layout: [t0-lo .. | t0-hi ..] so each half's gather
    # region is contiguous across the group (fewer dma_gather calls).
    # sched.order[j] = (t, half) of the j-th chunk group in schedule order.
    import os
    PS = int(os.environ.get('K_PS', '2'))
    order = []           # flat chunk -> (t, half)
    pair_lo = []         # per group: (#lo chunks, #hi chunks)
    for p0 in range(0, NT, PS):
        ts = [t for t in range(p0, min(p0 + PS, NT))]
        nlo = sum(n_lo[t] for t in ts)
        nhi = sum(n_hi[t] for t in ts)
        pair_lo.append((nlo, nhi))
        for t in ts:
            order += [(t, 0)] * n_lo[t]
        for t in ts:
            order += [(t, 1)] * n_hi[t]
    sched.order = order
    sched.pair_lo = pair_lo
    sched.ps = PS
    # chunk offset of (t, half) within the flat schedule
    choff = {}
    off = 0
    for (t, half) in order:
        choff.setdefault((t, half), off)
        off += 1
    sched.choff = choff

    # per-core edge arrays in schedule order
    idx_h = []       # [128, total*8] int16  (gather idx, lo/hi-local rows)
    dstloc = []      # [128, total] f16      (tile-local dst or -1, p=edge%128)
    for k in range(NCORES):
        ih = np.zeros((128, sched.total * 8), np.int16)
        dl = np.full((128, sched.total), -1.0, np.float16)
        for t in range(NT):
            for half, nch in ((0, n_lo[t]), (1, n_hi[t])):
                if nch == 0:
                    continue
                off = choff[(t, half)]
                s_t, dl_t = per[k][t][half]
                ne = nch * 128
                sp = _pad128(np.concatenate([s_t, np.zeros(ne - len(s_t),
                                                           np.int64)]), 0)[:ne]
                sp[len(s_t):] = 0
                dlp = np.full(ne, -1.0, np.float32)
                dlp[:len(dl_t)] = (dl_t - t * 128).astype(np.float32)
                ih[:, off * 8:(off + nch) * 8] = _wrap_idx(sp)
                dl[:, off:off + nch] = dlp.reshape(nch, 128).T.astype(np.float16)
        idx_h.append(ih)
        dstloc.append(dl)

    # weights: [W | v_s | v_d]. The dense phase emits [h | a_src | a_dst];
    # [h | a_src] goes to the gather table row, a_dst to the local stash.
    def aug(W, a_s, a_d, C):
        v_s = np.einsum("fhc,hc->fh", W.reshape(-1, H, C), a_s)
        v_d = np.einsum("fhc,hc->fh", W.reshape(-1, H, C), a_d)
        return np.concatenate([W, v_s, v_d], axis=1).astype(np.float16)

    W1aug = aug(W1, att_src1, att_dst1, C1)              # [IN, F+8]
    W2aug = aug(W2, att_src2, att_dst2, C2)              # [F, F+8]
    W2aug_pk = W2aug.reshape(2, 128, F + 8).transpose(1, 0, 2).copy()

    xT16 = np.ascontiguousarray(x.T).astype(np.float16)  # [IN, N]

    iota16 = np.tile(np.arange(128, dtype=np.float16), (128, 1))
    ident16 = np.eye(128, dtype=np.float16)

    const = {
        "W1aug": W1aug, "W2aug": W2aug_pk,
        "iota16": iota16, "ident16": ident16,
        "b1rep": np.tile(b1.astype(np.float32), (128, 1)),
        "b2rep": np.tile(b2.astype(np.float32), (128, 1)),
        "f1brep": np.tile(ff1_b.astype(np.float32), (128, 1)),
        "f2brep": np.tile(ff2_b.astype(np.float32), (128, 1)),
        "ff1w16": ff1_w.astype(np.float16),
        "ff2w16": ff2_w.astype(np.float16),
    }

    in_maps = []
    for k in range(NCORES):
        m = dict(const)
        m.update({
            "xT16": np.ascontiguousarray(xT16[:, k * NSH:(k + 1) * NSH]),
            "idx_h": idx_h[k], "dstloc": dstloc[k],
        })
        in_maps.append(m)

    dims = dict(N=N, IN=IN, F=F, NSH=NSH, NT=NT, NSH_LO=NSH_LO,
                NSH_HI=NSH_HI, C2=C2, FH=ff1_w.shape[1])
    return in_maps, sched, dims


# ----------------------------------------------------------------------------
# device program
# ----------------------------------------------------------------------------

def _gather_split(nc, out_ap_fn, tab, idx_sb, n_chunks, elem, q0):
    """Emit dma_gather calls capped at 8 chunks (1024 idxs) each.
    out_ap_fn(c0, c1) -> output AP for chunk range; idx_sb indexed per chunk.
    single_packet=False for rows >=512B (measured 286 vs 55 GB/s); True is
    faster for 256B rows (145-173 GB/s)."""
    sp = elem * 2 < 512
    c0 = 0
    q = q0
    while c0 < n_chunks:
        c1 = min(c0 + 8, n_chunks)
        nc.gpsimd.dma_gather(
            out_ap_fn(c0, c1), tab, idx_sb[:, c0 * 8:c1 * 8],
            num_idxs=(c1 - c0) * 128, num_idxs_reg=(c1 - c0) * 128,
            elem_size=elem, queue_num=q % 4, single_packet=sp)
        q += 1
        c0 = c1


def _build(sched, dims):
    import os, hashlib
    PH = int(os.environ.get('K_PHASES', '6'))
    TAPS = int(os.environ.get('K_TAPS', '0'))
    SIM = int(os.environ.get('K_SIM', '0'))
    PS = sched.ps
    BUFS = int(os.environ.get('K_BUFS', '3'))
    # phase repeat counts (profiling aid: phase time = dE/(R-1))
    R1 = int(os.environ.get('K_R1', '1'))
    R2 = int(os.environ.get('K_R2', '1'))
    R3 = int(os.environ.get('K_R3', '1'))
    R4 = int(os.environ.get('K_R4', '1'))
    R5 = int(os.environ.get('K_R5', '1'))
    R6 = int(os.environ.get('K_R6', '1'))
    N, IN, F, NSH, NT = dims["N"], dims["IN"], dims["F"], dims["NSH"], dims["NT"]
    NSH_LO, NSH_HI = dims["NSH_LO"], dims["NSH_HI"]
    NLO_T, NHI_T = NSH_LO * NCORES, NSH_HI * NCORES
    C2, FH = dims["C2"], dims["FH"]
    FA = F + 8       # dense output: [h | a_src | a_dst]
    FT = F + 4       # table row payload: [h | a_src]
    FR = 384         # table row stride (SWDGE needs 256B-multiple rows)

    nc = bacc.Bacc("TRN2", target_bir_lowering=False,
                   num_devices=(1 if SIM else NCORES), num_swdge_queues=4)
    # The neuronx compile cache keys on the jit signature only (the embedded
    # BIR is not hashed), so two different programs with identical I/O would
    # alias to one NEFF. A content-named dummy input de-aliases them.
    with open(__file__, "rb") as _f:
        _salt = hashlib.sha256(
            _f.read() + repr((sched.n_lo, sched.n_hi, sched.order,
                              sorted(dims.items()),
                              PH, BUFS, R1, R2, R3, R4, R5, R6)).encode()
        ).hexdigest()[:16]
    nc.dram_tensor(f"salt_{_salt}", [1, 4], dt.float32, kind="ExternalInput")
    dims["salt_name"] = f"salt_{_salt}"
    nc._salt_name = f"salt_{_salt}"

    # inputs
    xT16 = nc.dram_tensor("xT16", [IN, NSH], dt.float16, kind="ExternalInput")
    W1aug = nc.dram_tensor("W1aug", [IN, FA], dt.float16, kind="ExternalInput")
    W2aug = nc.dram_tensor("W2aug", [128, 2, FA], dt.float16, kind="ExternalInput")
    iota_in = nc.dram_tensor("iota16", [128, 128], dt.float16, kind="ExternalInput")
    ident_in = nc.dram_tensor("ident16", [128, 128], dt.float16, kind="ExternalInput")
    b1rep = nc.dram_tensor("b1rep", [128, F], dt.float32, kind="ExternalInput")
    b2rep = nc.dram_tensor("b2rep", [128, C2], dt.float32, kind="ExternalInput")
    f1brep = nc.dram_tensor("f1brep", [128, FH], dt.float32, kind="ExternalInput")
    f2brep = nc.dram_tensor("f2brep", [128, 2], dt.float32, kind="ExternalInput")
    ff1w16 = nc.dram_tensor("ff1w16", [C2, FH], dt.float16, kind="ExternalInput")
    ff2w16 = nc.dram_tensor("ff2w16", [FH, 2], dt.float16, kind="ExternalInput")
    idx_h_d = nc.dram_tensor("idx_h", [128, sched.total * 8], dt.int16, kind="ExternalInput")
    dstloc_d = nc.dram_tensor("dstloc", [128, sched.total], dt.float16, kind="ExternalInput")

    out_d = nc.dram_tensor("out", [NSH, 2], dt.float32, kind="ExternalOutput")
    if TAPS:
        tap_t1 = nc.dram_tensor("tap_t1", [256, FT], dt.float16, kind="ExternalOutput")
        tap_ag1 = nc.dram_tensor("tap_ag1", [256, FT], dt.float16, kind="ExternalOutput")
        tap_h1 = nc.dram_tensor("tap_h1", [NSH, F], dt.float16, kind="ExternalOutput")
        tap_ag2 = nc.dram_tensor("tap_ag2", [256, FT], dt.float16, kind="ExternalOutput")
        tap_h2 = nc.dram_tensor("tap_h2", [NSH, C2], dt.float16, kind="ExternalOutput")

    with tile.TileContext(nc) as tc, ExitStack() as octx:
        # persistent pools
        dram = octx.enter_context(tc.tile_pool(name="dram", bufs=1, space="DRAM"))
        cpool = octx.enter_context(tc.tile_pool(name="const", bufs=1))
        stash = octx.enter_context(tc.tile_pool(name="stash", bufs=1))

        # DRAM tables ([h | a_src | 124-col pad] rows, 768B stride).
        # Each layer's table is exchanged as TWO per-shard-half AllGathers
        # (lo = local rows [0:NSH_LO) of every shard) so the lo-half edge
        # gathers can start while the hi half is still in flight.
        t1h_own = dram.tile([NSH, FR], dt.float16)
        t2h_own = dram.tile([NSH, FR], dt.float16)
        _aspace = {} if SIM else {"addr_space": "Shared"}
        t1lo = dram.tile([NLO_T, FR], dt.float16, **_aspace)
        t1hi = dram.tile([NHI_T, FR], dt.float16, **_aspace)
        t2lo = dram.tile([NLO_T, FR], dt.float16, **_aspace)
        t2hi = dram.tile([NHI_T, FR], dt.float16, **_aspace)

        # constants in SBUF
        iota16 = cpool.tile([128, 128], dt.float16)
        nc.sync.dma_start(iota16[:], iota_in[:])
        ident16 = cpool.tile([128, 128], dt.float16)
        nc.sync.dma_start(ident16[:], ident_in[:])
        w1a_sb = cpool.tile([IN, FA], dt.float16)
        nc.sync.dma_start(w1a_sb[:], W1aug[:])
        w2a_sb = cpool.tile([128, 2, FA], dt.float16)
        nc.sync.dma_start(w2a_sb[:], W2aug[:])
        b1_sb = cpool.tile([128, F], dt.float32)
        nc.sync.dma_start(b1_sb[:], b1rep[:])
        b2_sb = cpool.tile([128, C2], dt.float32)
        nc.sync.dma_start(b2_sb[:], b2rep[:])
        f1b_sb = cpool.tile([128, FH], dt.float32)
        nc.sync.dma_start(f1b_sb[:], f1brep[:])
        f2b_sb = cpool.tile([128, 2], dt.float32)
        nc.sync.dma_start(f2b_sb[:], f2brep[:])
        ff1_sb = cpool.tile([C2, FH], dt.float16)
        nc.sync.dma_start(ff1_sb[:], ff1w16[:])
        ff2_sb = cpool.tile([FH, 2], dt.float16)
        nc.sync.dma_start(ff2_sb[:], ff2w16[:])

        # layer-1 hidden transposed, kept in SBUF for the layer-2 dense
        h1T = stash.tile([128, 2, NT, 128], dt.float16)
        out_stage = stash.tile([128, NT, 2], dt.float32)
        # per-own-tile a_dst projections, SBUF-resident.
        # Zeroed first: the a_dst matmul contracts over all 128 partitions and
        # 0 * uninitialized-NaN would poison PSUM (partial last tile).
        n1stash = stash.tile([128, NT, H], dt.float16)
        nc.vector.memset(n1stash[:], 0.0)
        n2stash = stash.tile([128, NT, H], dt.float16)
        nc.vector.memset(n2stash[:], 0.0)

        # ------------------------------------------------------------------
        # phase A: sharded layer-1 dense -> t1h_own / t1n_own
        # ------------------------------------------------------------------
        for _rep1 in range(R1):
         with ExitStack() as ctx:
            xp = ctx.enter_context(tc.tile_pool(name="xp", bufs=2))
            pp = ctx.enter_context(tc.tile_pool(name="pp", bufs=4, space="PSUM"))
            sp = ctx.enter_context(tc.tile_pool(name="sp", bufs=2))

            G = 8
            t0 = 0
            while t0 < NT:
                g = min(G, NT - t0)
                rows_g = min(g * 128, NSH - t0 * 128)
                xs = xp.tile([IN, G * 128], dt.float16, tag="xs")
                nc.sync.dma_start(xs[:, 0:rows_g],
                                  xT16[:, t0 * 128:t0 * 128 + rows_g])
                hst = sp.tile([128, G, FT], dt.float16, tag="hst")
                for j in range(g):
                    t = t0 + j
                    rows = min(128, NSH - t * 128)
                    ps = pp.tile([128, FA], dt.float32, tag="ps")
                    nc.tensor.matmul(ps[0:rows, :], xs[:, j * 128:j * 128 + rows],
                                     w1a_sb[:], start=True, stop=True)
                    nc.scalar.activation(hst[0:rows, j, :], ps[0:rows, 0:FT],
                                         ACT.Copy)
                    nc.vector.tensor_copy(n1stash[0:rows, t, :],
                                          ps[0:rows, FT:FA])
                _wr_rows(nc, t1h_own, t0 * 128, rows_g, hst, FT)
                t0 += g

        # ------------------------------------------------------------------
        # exchange 1
        # ------------------------------------------------------------------
        if PH >= 2:
          for _rep2 in range(R2):
            if SIM:
                for k in range(NCORES):
                    nc.sync.dma_start(
                        t1lo[k * NSH_LO:(k + 1) * NSH_LO, :],
                        t1h_own[0:NSH_LO, :])
                for k in range(NCORES):
                    nc.sync.dma_start(
                        t1hi[k * NSH_HI:(k + 1) * NSH_HI, :],
                        t1h_own[NSH_LO:NSH, :])
            else:
                tl, th = t1lo, t1hi
                if _rep2 != R2 - 1:
                    tl = dram.tile([NLO_T, FR], dt.float16,
                                   addr_space="Shared", name=f"t1lr{_rep2}")
                    th = dram.tile([NHI_T, FR], dt.float16,
                                   addr_space="Shared", name=f"t1hr{_rep2}")
                nc.gpsimd.collective_compute(
                    "AllGather", OP.bypass,
                    replica_groups=[list(range(NCORES))],
                    ins=[t1h_own[0:NSH_LO, :].opt()], outs=[tl[:].opt()])
                nc.gpsimd.collective_compute(
                    "AllGather", OP.bypass,
                    replica_groups=[list(range(NCORES))],
                    ins=[t1h_own[NSH_LO:NSH, :].opt()], outs=[th[:].opt()])

        # ------------------------------------------------------------------
        # edge phases
        # ------------------------------------------------------------------
        def edge_phase(ctx, name, tab_lo, tab_hi, nstash, evict):
            # Tiles are processed in PAIRS: all elementwise stages run batched
            # over the pair's chunk range. Within a pair, chunks are ordered
            # [t0-lo | t1-lo | t0-hi | t1-hi] so each half's gather region is
            # contiguous. The transposed one-hot (for the a_dst matmul) is
            # built by PE-transposing oh; Activation evacuates PSUM->SBUF.
            # Software pipeline: FRONT(i) [loads, gathers, oh, ohT, psA] /
            # BACK(i-1) [alpha..fold, scatter] / EVICT(i-2). Engine queues
            # are in-order; without this, evict's PSUM read and alpha's psA
            # wait head-of-line-block the next group's independent work.
            ep = ctx.enter_context(tc.tile_pool(name=name + "e", bufs=BUFS))
            pp = ctx.enter_context(tc.tile_pool(name=name + "p", bufs=2, space="PSUM"))
            groups = []
            for ip, p0 in enumerate(range(0, NT, PS)):
                p1 = min(p0 + PS, NT)
                tiles = [t for t in range(p0, p1) if sched.ct[t] > 0]
                if not tiles:
                    continue
                b0 = sched.base[p0]
                gct = sched.base[p1] - b0
                ctile = [sched.order[b0 + j][0] for j in range(gct)]
                groups.append(dict(
                    ip=ip, b0=b0, gct=gct, tiles=tiles, ctile=ctile,
                    nlo=sched.pair_lo[ip][0], nhi=sched.pair_lo[ip][1],
                    first={t: min(j for j in range(gct) if ctile[j] == t)
                           for t in tiles},
                    last={t: max(j for j in range(gct) if ctile[j] == t)
                          for t in tiles}))

            def front(G):
                b0, gct = G["b0"], G["gct"]
                ixh = ep.tile([128, gct * 8], dt.int16, tag="ixh")
                nc.sync.dma_start(ixh[:], idx_h_d[:, b0 * 8:(b0 + gct) * 8])
                dl = ep.tile([128, gct], dt.float16, tag="dl")
                nc.sync.dma_start(dl[:], dstloc_d[:, b0:b0 + gct])

                g = G["g"] = ep.tile([128, gct, FR], dt.float16, tag="g",
                                     name="g")
                if G["nlo"]:
                    _gather_split(
                        nc, lambda a, b: g[:, a:b, :], tab_lo,
                        ixh[:, 0:G["nlo"] * 8], G["nlo"], FR, 0)
                if G["nhi"]:
                    _gather_split(
                        nc, lambda a, b, o=G["nlo"]: g[:, o + a:o + b, :],
                        tab_hi, ixh[:, G["nlo"] * 8:gct * 8], G["nhi"], FR, 2)

                # one-hot: oh[e, c, d] = (iota[e, d] == dl[e, c]).
                # dl2 pre-broadcast (last-dim pair) keeps every operand's
                # last dim packed 2-byte -> DVE 2x mode.
                dl2 = ep.tile([128, gct, 2], dt.float16, tag="dl2")
                nc.vector.tensor_copy(
                    dl2[:], dl[:].unsqueeze(2).broadcast_to([128, gct, 2]))
                oh = G["oh"] = ep.tile([128, gct, 128], dt.float16, tag="oh",
                                       name="oh")
                nc.vector.tensor_tensor(
                    oh[:].rearrange("p c (d e) -> p c d e", e=2),
                    iota16[:].rearrange("p (d e) -> p d e", e=2)
                        .unsqueeze(1).broadcast_to([128, gct, 64, 2]),
                    dl2[:].unsqueeze(2).broadcast_to([128, gct, 64, 2]),
                    op=OP.is_equal)

                # ohT via PE transpose (4 chunks per PSUM tile; Act evacuates)
                ohT = ep.tile([128, gct, 128], dt.float16, tag="ohT")
                for j0 in range(0, gct, 4):
                    j1 = min(j0 + 4, gct)
                    tp4 = pp.tile([128, 4, 128], dt.float16, tag="tp4", bufs=1)
                    for j in range(j0, j1):
                        nc.tensor.transpose(tp4[:, j - j0, :], oh[:, j, :],
                                            ident16[:])
                    nc.scalar.activation(ohT[:, j0:j1, :], tp4[:, 0:j1 - j0, :],
                                         ACT.Copy)

                # a_dst per edge: psA[e, (c)H+h] = nstash[dst_e, t(c), h]
                psA = G["psA"] = pp.tile([128, gct * H], dt.float32,
                                         tag="psA", bufs=2, name="psA")
                for j in range(gct):
                    nc.tensor.matmul(
                        psA[:, j * H:(j + 1) * H],
                        ohT[:, j, :], nstash[:, G["ctile"][j], :],
                        start=True, stop=True)

            def back(G):
                gct, g, psA = G["gct"], G["g"], G["psA"]
                # alpha = lrelu(a_src + a_dst); a_src rides in the gathered
                # row (cols F:F+H). ex = exp(alpha)
                alpha = ep.tile([128, gct, H], dt.float32, tag="alpha")
                nc.vector.tensor_tensor(
                    alpha[:], g[:, :, F:F + H],
                    psA[:].rearrange("p (c h) -> p c h", h=H),
                    op=OP.add)
                nc.vector.scalar_tensor_tensor(
                    alpha[:], alpha[:], float(NEG_SLOPE), alpha[:],
                    op0=OP.mult, op1=OP.max)
                # exp lands in g's a_src slot (already consumed by alpha) so
                # the scatter is ONE 260-col matmul per chunk -> a single
                # PSUM accumulation group (interleaved groups mis-accumulate)
                nc.scalar.activation(g[:, :, F:F + H], alpha[:], ACT.Exp)
                # fold in place: g *= ex (per-head broadcast); ex2 pre-pairs
                # the broadcast so all last dims stay packed -> DVE 2x.
                ex2 = ep.tile([128, gct, H, 2], dt.float16, tag="ex2")
                nc.vector.tensor_copy(
                    ex2[:], g[:, :, F:F + H].unsqueeze(3)
                    .broadcast_to([128, gct, H, 2]))
                nc.vector.tensor_tensor(
                    g[:, :, 0:F].rearrange("p c (h d e) -> p c h d e",
                                           h=H, e=2),
                    g[:, :, 0:F].rearrange("p c (h d e) -> p c h d e",
                                           h=H, e=2),
                    ex2[:].unsqueeze(3).broadcast_to(
                        [128, gct, H, F // H // 2, 2]),
                    op=OP.mult)
                # matmul-scatter per chunk: [messages | denominators] in one
                # accumulation group per tile
                G["pss"] = {}
                for t in G["tiles"]:
                    G["pss"][t] = pp.tile([128, F + H], dt.float32, tag="ps",
                                          bufs=2, name=f"ps_t{t}")
                for j in range(gct):
                    t = G["ctile"][j]
                    nc.tensor.matmul(
                        G["pss"][t][:, 0:F + H], G["oh"][:, j, :],
                        g[:, j, 0:F + H],
                        start=(j == G["first"][t]), stop=(j == G["last"][t]))

            def ev(G):
                for t in G["tiles"]:
                    evict(ep, pp, t, G["pss"][t])

            ng = len(groups)
            for i in range(ng + 2):
                if i < ng:
                    front(groups[i])
                if 0 <= i - 1 < ng:
                    back(groups[i - 1])
                if 0 <= i - 2 < ng:
                    ev(groups[i - 2])


        # ---- layer 1 evict: h1 = relu(agg/den + b1); build h1T ----
        def evict1(ep, pp, t, ps):
            rows = min(128, NSH - t * 128)
            rcp = ep.tile([128, H], dt.float32, tag="rcp")
            nc.vector.reciprocal(rcp[:], ps[:, F:F + H])
            pre = ep.tile([128, F], dt.float32, tag="pre")
            nc.vector.tensor_tensor(
                pre[:].rearrange("p (h d) -> p h d", h=H),
                ps[:, 0:F].rearrange("p (h d) -> p h d", h=H),
                rcp[:].unsqueeze(2).broadcast_to([128, H, F // H]), op=OP.mult)
            nc.vector.tensor_tensor(pre[:], pre[:], b1_sb[:], op=OP.add)
            h1r = ep.tile([128, F], dt.float16, tag="h1r")
            nc.scalar.activation(h1r[:], pre[:], ACT.Relu)
            if TAPS:
                nc.sync.dma_start(tap_h1[t * 128:t * 128 + rows, :],
                                  h1r[0:rows, :])
            tp = pp.tile([128, 4, 128], dt.float16, tag="tpe", bufs=1,
                         name="tp_ev1")
            for b in range(2):
                nc.tensor.transpose(tp[:, b, :], h1r[:, b * 128:(b + 1) * 128],
                                    ident16[:])
            nc.scalar.activation(h1T[:, :, t, :], tp[:, 0:2, :], ACT.Copy)

        if PH >= 3:
         for _rep3 in range(R3):
          with ExitStack() as ctx:
            edge_phase(ctx, "l1", t1lo[:], t1hi[:], n1stash, evict1)

        # ------------------------------------------------------------------
        # phase C: layer-2 dense on own rows -> t2h_own / t2n_own
        # ------------------------------------------------------------------
        if PH >= 4:
         for _rep4 in range(R4):
          with ExitStack() as ctx:
            cp = ctx.enter_context(tc.tile_pool(name="cp", bufs=2))
            pp = ctx.enter_context(tc.tile_pool(name="cpp", bufs=4, space="PSUM"))
            G = 8
            t0 = 0
            while t0 < NT:
                g = min(G, NT - t0)
                hst = cp.tile([128, G, FT], dt.float16, tag="hst")
                for j in range(g):
                    t = t0 + j
                    rows = min(128, NSH - t * 128)
                    ps = pp.tile([128, FA], dt.float32, tag="ps")
                    for b in range(2):
                        nc.tensor.matmul(ps[:], h1T[:, b, t, :], w2a_sb[:, b, :],
                                         start=(b == 0), stop=(b == 1))
                    nc.scalar.activation(hst[:, j, :], ps[:, 0:FT], ACT.Copy)
                    nc.vector.tensor_copy(n2stash[0:rows, t, :],
                                          ps[0:rows, FT:FA])
                rows_t = min(g * 128, NSH - t0 * 128)
                _wr_rows(nc, t2h_own, t0 * 128, rows_t, hst, FT)
                t0 += g

        # ------------------------------------------------------------------
        # exchange 2
        # ------------------------------------------------------------------
        if PH >= 5:
          for _rep5 in range(R5):
            if SIM:
                for k in range(NCORES):
                    nc.sync.dma_start(
                        t2lo[k * NSH_LO:(k + 1) * NSH_LO, :],
                        t2h_own[0:NSH_LO, :])
                for k in range(NCORES):
                    nc.sync.dma_start(
                        t2hi[k * NSH_HI:(k + 1) * NSH_HI, :],
                        t2h_own[NSH_LO:NSH, :])
            else:
                tl, th = t2lo, t2hi
                if _rep5 != R5 - 1:
                    tl = dram.tile([NLO_T, FR], dt.float16,
                                   addr_space="Shared", name=f"t2lr{_rep5}")
                    th = dram.tile([NHI_T, FR], dt.float16,
                                   addr_space="Shared", name=f"t2hr{_rep5}")
                nc.gpsimd.collective_compute(
                    "AllGather", OP.bypass,
                    replica_groups=[list(range(NCORES))],
                    ins=[t2h_own[0:NSH_LO, :].opt()], outs=[tl[:].opt()])
                nc.gpsimd.collective_compute(
                    "AllGather", OP.bypass,
                    replica_groups=[list(range(NCORES))],
                    ins=[t2h_own[NSH_LO:NSH, :].opt()], outs=[th[:].opt()])

        # ---- layer 2 evict: h2 = relu(mean_h(agg/den) + b2); FF head ----
        def evict2(ep, pp, t, ps):
            rows = min(128, NSH - t * 128)
            rcp = ep.tile([128, H], dt.float32, tag="rcp")
            nc.vector.reciprocal(rcp[:], ps[:, F:F + H])
            pre = ep.tile([128, H, C2], dt.float32, tag="pre")
            nc.vector.tensor_tensor(
                pre[:], ps[:, 0:F].rearrange("p (h d) -> p h d", h=H),
                rcp[:].unsqueeze(2).broadcast_to([128, H, C2]), op=OP.mult)
            red = ep.tile([128, C2], dt.float32, tag="red")
            nc.vector.tensor_reduce(red[:], pre[:].transpose([0, 2, 1]),
                                    axis=mybir.AxisListType.X, op=OP.add)
            nc.vector.scalar_tensor_tensor(red[:], red[:], 1.0 / H, b2_sb[:],
                                           op0=OP.mult, op1=OP.add)
            h2 = ep.tile([128, 128], dt.float16, tag="h2")
            nc.vector.memset(h2[:, C2:128], 0.0)
            nc.scalar.activation(h2[:, 0:C2], red[:], ACT.Relu)
            if TAPS:
                nc.sync.dma_start(tap_h2[t * 128:t * 128 + rows, :],
                                  h2[0:rows, 0:C2])
            # FF: out = relu(h2 @ ff1 + b1f) @ ff2 + b2f  (square transposes)
            tp = pp.tile([128, 4, 128], dt.float16, tag="tpe", bufs=1,
                         name="tp_ev2a")
            nc.tensor.transpose(tp[:, 0, :], h2[:], ident16[:])
            h2T = ep.tile([C2, 128], dt.float16, tag="h2T")
            nc.scalar.activation(h2T[:], tp[0:C2, 0, :], ACT.Copy)
            pf1 = pp.tile([128, FH], dt.float32, tag="tpe", bufs=1,
                          name="pf1")
            nc.tensor.matmul(pf1[:], h2T[:], ff1_sb[:], start=True, stop=True)
            f1p = ep.tile([128, FH], dt.float32, tag="f1p")
            nc.vector.tensor_tensor(f1p[:], pf1[:], f1b_sb[:], op=OP.add)
            f1 = ep.tile([128, 128], dt.float16, tag="f1")
            nc.vector.memset(f1[:, FH:128], 0.0)
            nc.scalar.activation(f1[:, 0:FH], f1p[:], ACT.Relu)
            tpf = pp.tile([128, 4, 128], dt.float16, tag="tpe", bufs=1,
                          name="tp_ev2b")
            nc.tensor.transpose(tpf[:, 0, :], f1[:], ident16[:])
            f1T = ep.tile([FH, 128], dt.float16, tag="f1T")
            nc.scalar.activation(f1T[:], tpf[0:FH, 0, :], ACT.Copy)
            pf2 = pp.tile([128, 2], dt.float32, tag="tpe", bufs=1,
                          name="pf2")
            nc.tensor.matmul(pf2[:], f1T[:], ff2_sb[:], start=True, stop=True)
            nc.vector.tensor_tensor(out_stage[:, t, :], pf2[:], f2b_sb[:],
                                    op=OP.add)

        if TAPS:
            nc.sync.dma_start(tap_t1[:], t1h_own[0:256, 0:FT])
            if PH >= 2:
                nc.sync.dma_start(tap_ag1[:], t1lo[NSH_LO:NSH_LO + 256, 0:FT])
            if PH >= 5:
                nc.sync.dma_start(tap_ag2[:], t2lo[NSH_LO:NSH_LO + 256, 0:FT])
        if PH >= 6:
         for _rep6 in range(R6):
          with ExitStack() as ctx:
            edge_phase(ctx, "l2", t2lo[:], t2hi[:], n2stash, evict2)

        # final output
        if PH < 6:
            nc.vector.memset(out_stage[:], 0.0)
        full = (NSH // 128) * 128
        if full:
            nc.sync.dma_start(
                out_d[0:full, :].rearrange("(t p) j -> p t j", p=128),
                out_stage[:, 0:full // 128, :])
        if NSH > full:
            nc.sync.dma_start(out_d[full:NSH, :],
                              out_stage[0:NSH - full, NT - 1, :])

    nc.compile()
    return nc


def _wr_rows(nc, dst, r0, rows, st, width, col0=0):
    """DMA staging [128, G, width] (rows r = g*128+p at [p, g]) to DRAM rows
    dst[r0:r0+rows, 0:width] (dst rows may be wider than the payload)."""
    g_full = rows // 128
    if g_full:
        nc.sync.dma_start(
            dst[r0:r0 + g_full * 128, 0:width]
            .rearrange("(g p) c -> p g c", p=128),
            st[:, col0:col0 + g_full, :])
    rem = rows - g_full * 128
    if rem:
        nc.sync.dma_start(dst[r0 + g_full * 128:r0 + rows, 0:width],
                          st[0:rem, col0 + g_full, :])


# ----------------------------------------------------------------------------
# entry point
# ----------------------------------------------------------------------------

_CACHE = {}


def kernel(x, edge_index, edge_attr, W1, att_src1, att_dst1, b1,
           W2, att_src2, att_dst2, b2, ff1_w, ff1_b, ff2_w, ff2_b):
    x = np.asarray(x, np.float32)
    edge_index = np.asarray(edge_index)
    args = [np.asarray(a, np.float32) for a in
            (W1, att_src1, att_dst1, b1, W2, att_src2, att_dst2, b2,
             ff1_w, ff1_b, ff2_w, ff2_b)]
    in_maps, sched, dims = _prep(x, edge_index, *args)
    key = (dims["N"], dims["IN"], tuple(sched.n_lo), tuple(sched.n_hi))
    if key not in _CACHE:
        _CACHE[key] = _build(sched, dims)
    nc = _CACHE[key]
    salt = np.zeros((1, 4), np.float32)
    for m in in_maps:
        m[nc._salt_name] = salt
    res = run_bass_kernel_spmd(nc, in_maps, list(range(NCORES))).results
    out = np.concatenate([res[k]["out"] for k in range(NCORES)], axis=0)
    return out.astype(np.float32)



# revision 68
# speedup vs baseline: 1.0013x; 1.0013x over previous
"""GAT (2-layer GATConv + FF head) on 8 Trainium2 NeuronCores.

Strategy (per sharding hint): nodes + incident edges partitioned by
destination across 8 cores; per-edge softmax/scatter local to the
destination shard via one-hot matmul-scatter into PSUM; small weights
replicated.

v3 layout: both layers are symmetric [sharded dense -> AllGather h-table ->
edge phase]. The dense transform uses an augmented weight [W | v_s | v_d] so
each node row carries [h | a_src | a_dst] in one matmul; [h | a_src] goes to
the gather table (768B-stride rows: SWDGE rows must be 256B multiples, so
260 payload cols pad to 384), a_dst to a local [NT,H] stash. The per-edge
gather then delivers a_src for free -- no per-edge DVE mult+reduce.

Edge phase per tile-group: gather rows, build the one-hot on DVE (packed-
pair views keep every operand 2-byte/packed-last-dim -> DVE 2x mode), get
the transposed one-hot via PE transpose + Act PSUM->SBUF copy, a_dst via a
one-hot matmul, alpha/lrelu on DVE, exp on Act straight into the row's
a_src slot so the scatter is ONE 260-col matmul per chunk (a single PSUM
accumulation group -- interleaved groups mis-accumulate on HW).

The emission is software-pipelined FRONT(i)/BACK(i-1)/EVICT(i-2): engine
queues execute in-order, so without it alpha's psA wait and evict's PSUM
read head-of-line-block the next group's independent work (~60% of the
edge-phase time).

Message path in fp16 (tables, gathered rows, attention weights, matmul
operands); accumulation in fp32 PSUM; per-edge logits in fp32.
"""
import sys
sys.path.insert(0, "/opt/trn_rl_repo")

import numpy as np
from contextlib import ExitStack

import concourse.bass as bass
import concourse.bacc as bacc
import concourse.tile as tile
import concourse.mybir as mybir
from concourse.bass_utils import run_bass_kernel_spmd

dt = mybir.dt
OP = mybir.AluOpType
ACT = mybir.ActivationFunctionType

NCORES = 8
H = 4
NEG_SLOPE = 0.2


# ----------------------------------------------------------------------------
# host-side prep
# ----------------------------------------------------------------------------

def _wrap_idx(idx):
    """Pack an index list into the SWDGE wrapped layout [128, n/16] int16:
    index i -> partition i%16 (replicated to all 8 16-partition groups),
    free offset i//16."""
    n = len(idx)
    assert n % 128 == 0
    out = np.zeros((128, n // 16), np.int16)
    a = np.asarray(idx, np.int16).reshape(n // 16, 16).T  # [16, n/16]
    for r in range(8):
        out[r * 16:(r + 1) * 16, :] = a
    return out


def _pad128(a, fill):
    n = len(a)
    m = ((n + 127) // 128) * 128
    return np.concatenate([a, np.full(m - n, fill, a.dtype)])


class Sched:
    """Static, core-uniform per-tile chunk schedule."""

    def __init__(self, n_lo, n_hi):
        self.n_lo = n_lo          # [NT] chunks for lo-half gathers
        self.n_hi = n_hi          # [NT] chunks for hi-half gathers
        self.ct = [a + b for a, b in zip(n_lo, n_hi)]
        self.base = np.concatenate([[0], np.cumsum(self.ct)]).astype(int)
        self.total = int(self.base[-1])  # total chunks per core


def _prep(x, edge_index, W1, att_src1, att_dst1, b1, W2, att_src2, att_dst2,
          b2, ff1_w, ff1_b, ff2_w, ff2_b):
    N, IN = x.shape
    F = W1.shape[1]               # 256
    C1 = F // H
    C2 = W2.shape[1] // H
    NSH = N // NCORES
    NT = (NSH + 127) // 128
    # lo/hi split is per-shard (local rows [0:NSH_LO) of each core's shard
    # go to the lo table) so each half can be AllGathered separately and
    # overlap with the other half's gathers. 128-aligned; both halves'
    # table sizes stay within int16 gather indexing.
    NSH_LO = ((NSH // 2) // 128) * 128
    NSH_HI = NSH - NSH_LO

    E = edge_index.shape[1]
    ar = np.arange(N, dtype=np.int64)
    src = np.concatenate([edge_index[0], ar])
    dst = np.concatenate([edge_index[1], ar])

    shard = dst // NSH
    dstloc_all = dst - shard * NSH

    # group edges per (core, tile, half)
    per = [[[None, None] for _ in range(NT)] for _ in range(NCORES)]
    for k in range(NCORES):
        m = shard == k
        s_k, dl_k = src[m], dstloc_all[m]
        t_k = dl_k // 128
        for t in range(NT):
            mt = t_k == t
            s_t, dl_t = s_k[mt], dl_k[mt]
            s_shard, s_loc = s_t // NSH, s_t % NSH
            lo = s_loc < NSH_LO
            per[k][t][0] = (s_shard[lo] * NSH_LO + s_loc[lo], dl_t[lo])
            per[k][t][1] = (s_shard[~lo] * NSH_HI + (s_loc[~lo] - NSH_LO),
                            dl_t[~lo])

    n_lo = [max((len(per[k][t][0][0]) + 127) // 128 for k in range(NCORES))
            for t in range(NT)]
    n_hi = [max((len(per[k][t][1][0]) + 127) // 128 for k in range(NCORES))
            for t in range(NT)]
    sched = Sched(n_lo, n_hi)

    # Per-group chunk layout: [t0-lo .. | t0-hi ..] so each half's gather
    # region is contiguous across the group (fewer dma_gather calls).
    # sched.order[j] = (t, half) of the j-th chunk group in schedule order.
    import os
    PS = int(os.environ.get('K_PS', '1'))
    order = []           # flat chunk -> (t, half)
    pair_lo = []         # per group: (#lo chunks, #hi chunks)
    for p0 in range(0, NT, PS):
        ts = [t for t in range(p0, min(p0 + PS, NT))]
        nlo = sum(n_lo[t] for t in ts)
        nhi = sum(n_hi[t] for t in ts)
        pair_lo.append((nlo, nhi))
        for t in ts:
            order += [(t, 0)] * n_lo[t]
        for t in ts:
            order += [(t, 1)] * n_hi[t]
    sched.order = order
    sched.pair_lo = pair_lo
    sched.ps = PS
    # chunk offset of (t, half) within the flat schedule
    choff = {}
    off = 0
    for (t, half) in order:
        choff.setdefault((t, half), off)
        off += 1
    sched.choff = choff

    # per-core edge arrays in schedule order
    idx_h = []       # [128, total*8] int16  (gather idx, lo/hi-local rows)
    dstloc = []      # [128, total] f16      (tile-local dst or -1, p=edge%128)
    for k in range(NCORES):
        ih = np.zeros((128, sched.total * 8), np.int16)
        dl = np.full((128, sched.total), -1.0, np.float16)
        for t in range(NT):
            for half, nch in ((0, n_lo[t]), (1, n_hi[t])):
                if nch == 0:
                    continue
                off = choff[(t, half)]
                s_t, dl_t = per[k][t][half]
                ne = nch * 128
                sp = _pad128(np.concatenate([s_t, np.zeros(ne - len(s_t),
                                                           np.int64)]), 0)[:ne]
                sp[len(s_t):] = 0
                dlp = np.full(ne, -1.0, np.float32)
                dlp[:len(dl_t)] = (dl_t - t * 128).astype(np.float32)
                ih[:, off * 8:(off + nch) * 8] = _wrap_idx(sp)
                dl[:, off:off + nch] = dlp.reshape(nch, 128).T.astype(np.float16)
        idx_h.append(ih)
        dstloc.append(dl)

    # weights: [W | v_s | v_d]. The dense phase emits [h | a_src | a_dst];
    # [h | a_src] goes to the gather table row, a_dst to the local stash.
    def aug(W, a_s, a_d, C):
        v_s = np.einsum("fhc,hc->fh", W.reshape(-1, H, C), a_s)
        v_d = np.einsum("fhc,hc->fh", W.reshape(-1, H, C), a_d)
        return np.concatenate([W, v_s, v_d], axis=1).astype(np.float16)

    W1aug = aug(W1, att_src1, att_dst1, C1)              # [IN, F+8]
    W2aug = aug(W2, att_src2, att_dst2, C2)              # [F, F+8]
    W2aug_pk = W2aug.reshape(2, 128, F + 8).transpose(1, 0, 2).copy()

    xT16 = np.ascontiguousarray(x.T).astype(np.float16)  # [IN, N]

    iota16 = np.tile(np.arange(128, dtype=np.float16), (128, 1))
    ident16 = np.eye(128, dtype=np.float16)

    const = {
        "W1aug": W1aug, "W2aug": W2aug_pk,
        "iota16": iota16, "ident16": ident16,
        "b1rep": np.tile(b1.astype(np.float32), (128, 1)),
        "b2rep": np.tile(b2.astype(np.float32), (128, 1)),
        "f1brep": np.tile(ff1_b.astype(np.float32), (128, 1)),
        "f2brep": np.tile(ff2_b.astype(np.float32), (128, 1)),
        "ff1w16": ff1_w.astype(np.float16),
        "ff2w16": ff2_w.astype(np.float16),
    }

    in_maps = []
    for k in range(NCORES):
        m = dict(const)
        m.update({
            "xT16": np.ascontiguousarray(xT16[:, k * NSH:(k + 1) * NSH]),
            "idx_h": idx_h[k], "dstloc": dstloc[k],
        })
        in_maps.append(m)

    dims = dict(N=N, IN=IN, F=F, NSH=NSH, NT=NT, NSH_LO=NSH_LO,
                NSH_HI=NSH_HI, C2=C2, FH=ff1_w.shape[1])
    return in_maps, sched, dims


# ----------------------------------------------------------------------------
# device program
# ----------------------------------------------------------------------------

def _gather_split(nc, out_ap_fn, tab, idx_sb, n_chunks, elem, q0):
    """Emit dma_gather calls capped at 8 chunks (1024 idxs) each.
    out_ap_fn(c0, c1) -> output AP for chunk range; idx_sb indexed per chunk.
    single_packet=False for rows >=512B (measured 286 vs 55 GB/s); True is
    faster for 256B rows (145-173 GB/s)."""
    sp = elem * 2 < 512
    c0 = 0
    q = q0
    while c0 < n_chunks:
        c1 = min(c0 + 8, n_chunks)
        nc.gpsimd.dma_gather(
            out_ap_fn(c0, c1), tab, idx_sb[:, c0 * 8:c1 * 8],
            num_idxs=(c1 - c0) * 128, num_idxs_reg=(c1 - c0) * 128,
            elem_size=elem, queue_num=q % 4, single_packet=sp)
        q += 1
        c0 = c1


def _build(sched, dims):
    import os, hashlib
    PH = int(os.environ.get('K_PHASES', '6'))
    TAPS = int(os.environ.get('K_TAPS', '0'))
    SIM = int(os.environ.get('K_SIM', '0'))
    PS = sched.ps
    BUFS = int(os.environ.get('K_BUFS', '6'))
    DLAG = int(os.environ.get('K_D', '2'))
    # phase repeat counts (profiling aid: phase time = dE/(R-1))
    R1 = int(os.environ.get('K_R1', '1'))
    R2 = int(os.environ.get('K_R2', '1'))
    R3 = int(os.environ.get('K_R3', '1'))
    R4 = int(os.environ.get('K_R4', '1'))
    R5 = int(os.environ.get('K_R5', '1'))
    R6 = int(os.environ.get('K_R6', '1'))
    N, IN, F, NSH, NT = dims["N"], dims["IN"], dims["F"], dims["NSH"], dims["NT"]
    NSH_LO, NSH_HI = dims["NSH_LO"], dims["NSH_HI"]
    NLO_T, NHI_T = NSH_LO * NCORES, NSH_HI * NCORES
    C2, FH = dims["C2"], dims["FH"]
    FA = F + 8       # dense output: [h | a_src | a_dst]
    FT = F + 4       # table row payload: [h | a_src]
    FR = 384         # table row stride (SWDGE needs 256B-multiple rows)

    nc = bacc.Bacc("TRN2", target_bir_lowering=False,
                   num_devices=(1 if SIM else NCORES), num_swdge_queues=4)
    # The neuronx compile cache keys on the jit signature only (the embedded
    # BIR is not hashed), so two different programs with identical I/O would
    # alias to one NEFF. A content-named dummy input de-aliases them.
    with open(__file__, "rb") as _f:
        _salt = hashlib.sha256(
            _f.read() + repr((sched.n_lo, sched.n_hi, sched.order,
                              sorted(dims.items()),
                              PH, BUFS, DLAG, R1, R2, R3, R4, R5, R6)).encode()
        ).hexdigest()[:16]
    nc.dram_tensor(f"salt_{_salt}", [1, 4], dt.float32, kind="ExternalInput")
    dims["salt_name"] = f"salt_{_salt}"
    nc._salt_name = f"salt_{_salt}"

    # inputs
    xT16 = nc.dram_tensor("xT16", [IN, NSH], dt.float16, kind="ExternalInput")
    W1aug = nc.dram_tensor("W1aug", [IN, FA], dt.float16, kind="ExternalInput")
    W2aug = nc.dram_tensor("W2aug", [128, 2, FA], dt.float16, kind="ExternalInput")
    iota_in = nc.dram_tensor("iota16", [128, 128], dt.float16, kind="ExternalInput")
    ident_in = nc.dram_tensor("ident16", [128, 128], dt.float16, kind="ExternalInput")
    b1rep = nc.dram_tensor("b1rep", [128, F], dt.float32, kind="ExternalInput")
    b2rep = nc.dram_tensor("b2rep", [128, C2], dt.float32, kind="ExternalInput")
    f1brep = nc.dram_tensor("f1brep", [128, FH], dt.float32, kind="ExternalInput")
    f2brep = nc.dram_tensor("f2brep", [128, 2], dt.float32, kind="ExternalInput")
    ff1w16 = nc.dram_tensor("ff1w16", [C2, FH], dt.float16, kind="ExternalInput")
    ff2w16 = nc.dram_tensor("ff2w16", [FH, 2], dt.float16, kind="ExternalInput")
    idx_h_d = nc.dram_tensor("idx_h", [128, sched.total * 8], dt.int16, kind="ExternalInput")
    dstloc_d = nc.dram_tensor("dstloc", [128, sched.total], dt.float16, kind="ExternalInput")

    out_d = nc.dram_tensor("out", [NSH, 2], dt.float32, kind="ExternalOutput")
    if TAPS:
        tap_t1 = nc.dram_tensor("tap_t1", [256, FT], dt.float16, kind="ExternalOutput")
        tap_ag1 = nc.dram_tensor("tap_ag1", [256, FT], dt.float16, kind="ExternalOutput")
        tap_h1 = nc.dram_tensor("tap_h1", [NSH, F], dt.float16, kind="ExternalOutput")
        tap_ag2 = nc.dram_tensor("tap_ag2", [256, FT], dt.float16, kind="ExternalOutput")
        tap_h2 = nc.dram_tensor("tap_h2", [NSH, C2], dt.float16, kind="ExternalOutput")

    with tile.TileContext(nc) as tc, ExitStack() as octx:
        # persistent pools
        dram = octx.enter_context(tc.tile_pool(name="dram", bufs=1, space="DRAM"))
        cpool = octx.enter_context(tc.tile_pool(name="const", bufs=1))
        stash = octx.enter_context(tc.tile_pool(name="stash", bufs=1))

        # DRAM tables ([h | a_src | 124-col pad] rows, 768B stride).
        # Each layer's table is exchanged as TWO per-shard-half AllGathers
        # (lo = local rows [0:NSH_LO) of every shard) so the lo-half edge
        # gathers can start while the hi half is still in flight.
        t1h_own = dram.tile([NSH, FR], dt.float16)
        t2h_own = dram.tile([NSH, FR], dt.float16)
        _aspace = {} if SIM else {"addr_space": "Shared"}
        t1lo = dram.tile([NLO_T, FR], dt.float16, **_aspace)
        t1hi = dram.tile([NHI_T, FR], dt.float16, **_aspace)
        t2lo = dram.tile([NLO_T, FR], dt.float16, **_aspace)
        t2hi = dram.tile([NHI_T, FR], dt.float16, **_aspace)

        # constants in SBUF
        iota16 = cpool.tile([128, 128], dt.float16)
        nc.sync.dma_start(iota16[:], iota_in[:])
        ident16 = cpool.tile([128, 128], dt.float16)
        nc.sync.dma_start(ident16[:], ident_in[:])
        w1a_sb = cpool.tile([IN, FA], dt.float16)
        nc.sync.dma_start(w1a_sb[:], W1aug[:])
        w2a_sb = cpool.tile([128, 2, FA], dt.float16)
        nc.sync.dma_start(w2a_sb[:], W2aug[:])
        b1_sb = cpool.tile([128, F], dt.float32)
        nc.sync.dma_start(b1_sb[:], b1rep[:])
        b2_sb = cpool.tile([128, C2], dt.float32)
        nc.sync.dma_start(b2_sb[:], b2rep[:])
        f1b_sb = cpool.tile([128, FH], dt.float32)
        nc.sync.dma_start(f1b_sb[:], f1brep[:])
        f2b_sb = cpool.tile([128, 2], dt.float32)
        nc.sync.dma_start(f2b_sb[:], f2brep[:])
        ff1_sb = cpool.tile([C2, FH], dt.float16)
        nc.sync.dma_start(ff1_sb[:], ff1w16[:])
        ff2_sb = cpool.tile([FH, 2], dt.float16)
        nc.sync.dma_start(ff2_sb[:], ff2w16[:])

        # layer-1 hidden transposed, kept in SBUF for the layer-2 dense
        h1T = stash.tile([128, 2, NT, 128], dt.float16)
        out_stage = stash.tile([128, NT, 2], dt.float32)
        # per-own-tile a_dst projections, SBUF-resident.
        # Zeroed first: the a_dst matmul contracts over all 128 partitions and
        # 0 * uninitialized-NaN would poison PSUM (partial last tile).
        n1stash = stash.tile([128, NT, H], dt.float16)
        nc.vector.memset(n1stash[:], 0.0)
        n2stash = stash.tile([128, NT, H], dt.float16)
        nc.vector.memset(n2stash[:], 0.0)

        # ------------------------------------------------------------------
        # phase A: sharded layer-1 dense -> t1h_own / t1n_own
        # ------------------------------------------------------------------
        for _rep1 in range(R1):
         with ExitStack() as ctx:
            xp = ctx.enter_context(tc.tile_pool(name="xp", bufs=2))
            pp = ctx.enter_context(tc.tile_pool(name="pp", bufs=4, space="PSUM"))
            sp = ctx.enter_context(tc.tile_pool(name="sp", bufs=2))

            G = 8
            t0 = 0
            while t0 < NT:
                g = min(G, NT - t0)
                rows_g = min(g * 128, NSH - t0 * 128)
                xs = xp.tile([IN, G * 128], dt.float16, tag="xs")
                nc.sync.dma_start(xs[:, 0:rows_g],
                                  xT16[:, t0 * 128:t0 * 128 + rows_g])
                hst = sp.tile([128, G, FT], dt.float16, tag="hst")
                for j in range(g):
                    t = t0 + j
                    rows = min(128, NSH - t * 128)
                    ps = pp.tile([128, FA], dt.float32, tag="ps")
                    nc.tensor.matmul(ps[0:rows, :], xs[:, j * 128:j * 128 + rows],
                                     w1a_sb[:], start=True, stop=True)
                    nc.scalar.activation(hst[0:rows, j, :], ps[0:rows, 0:FT],
                                         ACT.Copy)
                    nc.vector.tensor_copy(n1stash[0:rows, t, :],
                                          ps[0:rows, FT:FA])
                _wr_rows(nc, t1h_own, t0 * 128, rows_g, hst, FT)
                t0 += g

        # ------------------------------------------------------------------
        # exchange 1
        # ------------------------------------------------------------------
        if PH >= 2:
          for _rep2 in range(R2):
            if SIM:
                for k in range(NCORES):
                    nc.sync.dma_start(
                        t1lo[k * NSH_LO:(k + 1) * NSH_LO, :],
                        t1h_own[0:NSH_LO, :])
                for k in range(NCORES):
                    nc.sync.dma_start(
                        t1hi[k * NSH_HI:(k + 1) * NSH_HI, :],
                        t1h_own[NSH_LO:NSH, :])
            else:
                tl, th = t1lo, t1hi
                if _rep2 != R2 - 1:
                    tl = dram.tile([NLO_T, FR], dt.float16,
                                   addr_space="Shared", name=f"t1lr{_rep2}")
                    th = dram.tile([NHI_T, FR], dt.float16,
                                   addr_space="Shared", name=f"t1hr{_rep2}")
                nc.gpsimd.collective_compute(
                    "AllGather", OP.bypass,
                    replica_groups=[list(range(NCORES))],
                    ins=[t1h_own[0:NSH_LO, :].opt()], outs=[tl[:].opt()])
                nc.gpsimd.collective_compute(
                    "AllGather", OP.bypass,
                    replica_groups=[list(range(NCORES))],
                    ins=[t1h_own[NSH_LO:NSH, :].opt()], outs=[th[:].opt()])

        # ------------------------------------------------------------------
        # edge phases
        # ------------------------------------------------------------------
        def edge_phase(ctx, name, tab_lo, tab_hi, nstash, evict):
            # Tiles are processed in PAIRS: all elementwise stages run batched
            # over the pair's chunk range. Within a pair, chunks are ordered
            # [t0-lo | t1-lo | t0-hi | t1-hi] so each half's gather region is
            # contiguous. The transposed one-hot (for the a_dst matmul) is
            # built by PE-transposing oh; Activation evacuates PSUM->SBUF.
            # Software pipeline: FRONT(i) [loads, gathers, oh, ohT, psA] /
            # BACK(i-1) [alpha..fold, scatter] / EVICT(i-2). Engine queues
            # are in-order; without this, evict's PSUM read and alpha's psA
            # wait head-of-line-block the next group's independent work.
            ep = ctx.enter_context(tc.tile_pool(name=name + "e", bufs=BUFS))
            pp = ctx.enter_context(tc.tile_pool(name=name + "p", bufs=2, space="PSUM"))
            groups = []
            for ip, p0 in enumerate(range(0, NT, PS)):
                p1 = min(p0 + PS, NT)
                tiles = [t for t in range(p0, p1) if sched.ct[t] > 0]
                if not tiles:
                    continue
                b0 = sched.base[p0]
                gct = sched.base[p1] - b0
                ctile = [sched.order[b0 + j][0] for j in range(gct)]
                groups.append(dict(
                    ip=ip, b0=b0, gct=gct, tiles=tiles, ctile=ctile,
                    nlo=sched.pair_lo[ip][0], nhi=sched.pair_lo[ip][1],
                    first={t: min(j for j in range(gct) if ctile[j] == t)
                           for t in tiles},
                    last={t: max(j for j in range(gct) if ctile[j] == t)
                          for t in tiles}))

            def front(G):
                b0, gct = G["b0"], G["gct"]
                ixh = ep.tile([128, gct * 8], dt.int16, tag="ixh")
                nc.sync.dma_start(ixh[:], idx_h_d[:, b0 * 8:(b0 + gct) * 8])
                dl = ep.tile([128, gct], dt.float16, tag="dl")
                nc.sync.dma_start(dl[:], dstloc_d[:, b0:b0 + gct])

                g = G["g"] = ep.tile([128, gct, FR], dt.float16, tag="g",
                                     name="g")
                if G["nlo"]:
                    _gather_split(
                        nc, lambda a, b: g[:, a:b, :], tab_lo,
                        ixh[:, 0:G["nlo"] * 8], G["nlo"], FR, 0)
                if G["nhi"]:
                    _gather_split(
                        nc, lambda a, b, o=G["nlo"]: g[:, o + a:o + b, :],
                        tab_hi, ixh[:, G["nlo"] * 8:gct * 8], G["nhi"], FR, 2)

                # one-hot: oh[e, c, d] = (iota[e, d] == dl[e, c]).
                # dl2 pre-broadcast (last-dim pair) keeps every operand's
                # last dim packed 2-byte -> DVE 2x mode.
                dl2 = ep.tile([128, gct, 2], dt.float16, tag="dl2")
                nc.vector.tensor_copy(
                    dl2[:], dl[:].unsqueeze(2).broadcast_to([128, gct, 2]))
                oh = G["oh"] = ep.tile([128, gct, 128], dt.float16, tag="oh",
                                       name="oh")
                nc.vector.tensor_tensor(
                    oh[:].rearrange("p c (d e) -> p c d e", e=2),
                    iota16[:].rearrange("p (d e) -> p d e", e=2)
                        .unsqueeze(1).broadcast_to([128, gct, 64, 2]),
                    dl2[:].unsqueeze(2).broadcast_to([128, gct, 64, 2]),
                    op=OP.is_equal)

                # ohT via PE transpose (4 chunks per PSUM tile; Act evacuates)
                ohT = ep.tile([128, gct, 128], dt.float16, tag="ohT")
                for j0 in range(0, gct, 4):
                    j1 = min(j0 + 4, gct)
                    tp4 = pp.tile([128, 4, 128], dt.float16, tag="tp4", bufs=1)
                    for j in range(j0, j1):
                        nc.tensor.transpose(tp4[:, j - j0, :], oh[:, j, :],
                                            ident16[:])
                    nc.scalar.activation(ohT[:, j0:j1, :], tp4[:, 0:j1 - j0, :],
                                         ACT.Copy)

                # a_dst per edge: psA[e, (c)H+h] = nstash[dst_e, t(c), h]
                psA = G["psA"] = pp.tile([128, gct * H], dt.float32,
                                         tag="psA", bufs=DLAG + 1,
                                         name="psA")
                for j in range(gct):
                    nc.tensor.matmul(
                        psA[:, j * H:(j + 1) * H],
                        ohT[:, j, :], nstash[:, G["ctile"][j], :],
                        start=True, stop=True)

            def back(G):
                gct, g, psA = G["gct"], G["g"], G["psA"]
                # alpha = lrelu(a_src + a_dst); a_src rides in the gathered
                # row (cols F:F+H). ex = exp(alpha)
                alpha = ep.tile([128, gct, H], dt.float32, tag="alpha")
                nc.vector.tensor_tensor(
                    alpha[:], g[:, :, F:F + H],
                    psA[:].rearrange("p (c h) -> p c h", h=H),
                    op=OP.add)
                nc.vector.scalar_tensor_tensor(
                    alpha[:], alpha[:], float(NEG_SLOPE), alpha[:],
                    op0=OP.mult, op1=OP.max)
                # exp lands in g's a_src slot (already consumed by alpha) so
                # the scatter is ONE 260-col matmul per chunk -> a single
                # PSUM accumulation group (interleaved groups mis-accumulate)
                nc.scalar.activation(g[:, :, F:F + H], alpha[:], ACT.Exp)
                # fold in place: g *= ex (per-head broadcast); ex2 pre-pairs
                # the broadcast so all last dims stay packed -> DVE 2x.
                ex2 = ep.tile([128, gct, H, 2], dt.float16, tag="ex2")
                nc.vector.tensor_copy(
                    ex2[:], g[:, :, F:F + H].unsqueeze(3)
                    .broadcast_to([128, gct, H, 2]))
                nc.vector.tensor_tensor(
                    g[:, :, 0:F].rearrange("p c (h d e) -> p c h d e",
                                           h=H, e=2),
                    g[:, :, 0:F].rearrange("p c (h d e) -> p c h d e",
                                           h=H, e=2),
                    ex2[:].unsqueeze(3).broadcast_to(
                        [128, gct, H, F // H // 2, 2]),
                    op=OP.mult)
                # matmul-scatter per chunk: [messages | denominators] in one
                # accumulation group per tile
                G["pss"] = {}
                for t in G["tiles"]:
                    G["pss"][t] = pp.tile([128, F + H], dt.float32, tag="ps",
                                          bufs=DLAG + 1, name=f"ps_t{t}")
                for j in range(gct):
                    t = G["ctile"][j]
                    nc.tensor.matmul(
                        G["pss"][t][:, 0:F + H], G["oh"][:, j, :],
                        g[:, j, 0:F + H],
                        start=(j == G["first"][t]), stop=(j == G["last"][t]))

            def ev(G):
                for t in G["tiles"]:
                    evict(ep, pp, t, G["pss"][t])

            ng = len(groups)
            for i in range(ng + 2 * DLAG):
                if i < ng:
                    front(groups[i])
                if 0 <= i - DLAG < ng:
                    back(groups[i - DLAG])
                if 0 <= i - 2 * DLAG < ng:
                    ev(groups[i - 2 * DLAG])


        # ---- layer 1 evict: h1 = relu(agg/den + b1); build h1T ----
        def evict1(ep, pp, t, ps):
            rows = min(128, NSH - t * 128)
            rcp = ep.tile([128, H], dt.float32, tag="rcp")
            nc.vector.reciprocal(rcp[:], ps[:, F:F + H])
            pre = ep.tile([128, F], dt.float32, tag="pre")
            nc.vector.tensor_tensor(
                pre[:].rearrange("p (h d) -> p h d", h=H),
                ps[:, 0:F].rearrange("p (h d) -> p h d", h=H),
                rcp[:].unsqueeze(2).broadcast_to([128, H, F // H]), op=OP.mult)
            nc.vector.tensor_tensor(pre[:], pre[:], b1_sb[:], op=OP.add)
            h1r = ep.tile([128, F], dt.float16, tag="h1r")
            nc.scalar.activation(h1r[:], pre[:], ACT.Relu)
            if TAPS:
                nc.sync.dma_start(tap_h1[t * 128:t * 128 + rows, :],
                                  h1r[0:rows, :])
            tp = pp.tile([128, 4, 128], dt.float16, tag="tpe", bufs=1,
                         name="tp_ev1")
            for b in range(2):
                nc.tensor.transpose(tp[:, b, :], h1r[:, b * 128:(b + 1) * 128],
                                    ident16[:])
            nc.scalar.activation(h1T[:, :, t, :], tp[:, 0:2, :], ACT.Copy)

        if PH >= 3:
         for _rep3 in range(R3):
          with ExitStack() as ctx:
            edge_phase(ctx, "l1", t1lo[:], t1hi[:], n1stash, evict1)

        # ------------------------------------------------------------------
        # phase C: layer-2 dense on own rows -> t2h_own / t2n_own
        # ------------------------------------------------------------------
        if PH >= 4:
         for _rep4 in range(R4):
          with ExitStack() as ctx:
            cp = ctx.enter_context(tc.tile_pool(name="cp", bufs=2))
            pp = ctx.enter_context(tc.tile_pool(name="cpp", bufs=4, space="PSUM"))
            G = 8
            t0 = 0
            while t0 < NT:
                g = min(G, NT - t0)
                hst = cp.tile([128, G, FT], dt.float16, tag="hst")
                for j in range(g):
                    t = t0 + j
                    rows = min(128, NSH - t * 128)
                    ps = pp.tile([128, FA], dt.float32, tag="ps")
                    for b in range(2):
                        nc.tensor.matmul(ps[:], h1T[:, b, t, :], w2a_sb[:, b, :],
                                         start=(b == 0), stop=(b == 1))
                    nc.scalar.activation(hst[:, j, :], ps[:, 0:FT], ACT.Copy)
                    nc.vector.tensor_copy(n2stash[0:rows, t, :],
                                          ps[0:rows, FT:FA])
                rows_t = min(g * 128, NSH - t0 * 128)
                _wr_rows(nc, t2h_own, t0 * 128, rows_t, hst, FT)
                t0 += g

        # ------------------------------------------------------------------
        # exchange 2
        # ------------------------------------------------------------------
        if PH >= 5:
          for _rep5 in range(R5):
            if SIM:
                for k in range(NCORES):
                    nc.sync.dma_start(
                        t2lo[k * NSH_LO:(k + 1) * NSH_LO, :],
                        t2h_own[0:NSH_LO, :])
                for k in range(NCORES):
                    nc.sync.dma_start(
                        t2hi[k * NSH_HI:(k + 1) * NSH_HI, :],
                        t2h_own[NSH_LO:NSH, :])
            else:
                tl, th = t2lo, t2hi
                if _rep5 != R5 - 1:
                    tl = dram.tile([NLO_T, FR], dt.float16,
                                   addr_space="Shared", name=f"t2lr{_rep5}")
                    th = dram.tile([NHI_T, FR], dt.float16,
                                   addr_space="Shared", name=f"t2hr{_rep5}")
                nc.gpsimd.collective_compute(
                    "AllGather", OP.bypass,
                    replica_groups=[list(range(NCORES))],
                    ins=[t2h_own[0:NSH_LO, :].opt()], outs=[tl[:].opt()])
                nc.gpsimd.collective_compute(
                    "AllGather", OP.bypass,
                    replica_groups=[list(range(NCORES))],
                    ins=[t2h_own[NSH_LO:NSH, :].opt()], outs=[th[:].opt()])

        # ---- layer 2 evict: h2 = relu(mean_h(agg/den) + b2); FF head ----
        def evict2(ep, pp, t, ps):
            rows = min(128, NSH - t * 128)
            rcp = ep.tile([128, H], dt.float32, tag="rcp")
            nc.vector.reciprocal(rcp[:], ps[:, F:F + H])
            pre = ep.tile([128, H, C2], dt.float32, tag="pre")
            nc.vector.tensor_tensor(
                pre[:], ps[:, 0:F].rearrange("p (h d) -> p h d", h=H),
                rcp[:].unsqueeze(2).broadcast_to([128, H, C2]), op=OP.mult)
            red = ep.tile([128, C2], dt.float32, tag="red")
            nc.vector.tensor_reduce(red[:], pre[:].transpose([0, 2, 1]),
                                    axis=mybir.AxisListType.X, op=OP.add)
            nc.vector.scalar_tensor_tensor(red[:], red[:], 1.0 / H, b2_sb[:],
                                           op0=OP.mult, op1=OP.add)
            h2 = ep.tile([128, 128], dt.float16, tag="h2")
            nc.vector.memset(h2[:, C2:128], 0.0)
            nc.scalar.activation(h2[:, 0:C2], red[:], ACT.Relu)
            if TAPS:
                nc.sync.dma_start(tap_h2[t * 128:t * 128 + rows, :],
                                  h2[0:rows, 0:C2])
            # FF: out = relu(h2 @ ff1 + b1f) @ ff2 + b2f  (square transposes)
            tp = pp.tile([128, 4, 128], dt.float16, tag="tpe", bufs=1,
                         name="tp_ev2a")
            nc.tensor.transpose(tp[:, 0, :], h2[:], ident16[:])
            h2T = ep.tile([C2, 128], dt.float16, tag="h2T")
            nc.scalar.activation(h2T[:], tp[0:C2, 0, :], ACT.Copy)
            pf1 = pp.tile([128, FH], dt.float32, tag="tpe", bufs=1,
                          name="pf1")
            nc.tensor.matmul(pf1[:], h2T[:], ff1_sb[:], start=True, stop=True)
            f1p = ep.tile([128, FH], dt.float32, tag="f1p")
            nc.vector.tensor_tensor(f1p[:], pf1[:], f1b_sb[:], op=OP.add)
            f1 = ep.tile([128, 128], dt.float16, tag="f1")
            nc.vector.memset(f1[:, FH:128], 0.0)
            nc.scalar.activation(f1[:, 0:FH], f1p[:], ACT.Relu)
            tpf = pp.tile([128, 4, 128], dt.float16, tag="tpe", bufs=1,
                          name="tp_ev2b")
            nc.tensor.transpose(tpf[:, 0, :], f1[:], ident16[:])
            f1T = ep.tile([FH, 128], dt.float16, tag="f1T")
            nc.scalar.activation(f1T[:], tpf[0:FH, 0, :], ACT.Copy)
            pf2 = pp.tile([128, 2], dt.float32, tag="tpe", bufs=1,
                          name="pf2")
            nc.tensor.matmul(pf2[:], f1T[:], ff2_sb[:], start=True, stop=True)
            nc.vector.tensor_tensor(out_stage[:, t, :], pf2[:], f2b_sb[:],
                                    op=OP.add)

        if TAPS:
            nc.sync.dma_start(tap_t1[:], t1h_own[0:256, 0:FT])
            if PH >= 2:
                nc.sync.dma_start(tap_ag1[:], t1lo[NSH_LO:NSH_LO + 256, 0:FT])
            if PH >= 5:
                nc.sync.dma_start(tap_ag2[:], t2lo[NSH_LO:NSH_LO + 256, 0:FT])
        if PH >= 6:
         for _rep6 in range(R6):
          with ExitStack() as ctx:
            edge_phase(ctx, "l2", t2lo[:], t2hi[:], n2stash, evict2)

        # final output
        if PH < 6:
            nc.vector.memset(out_stage[:], 0.0)
        full = (NSH // 128) * 128
        if full:
            nc.sync.dma_start(
                out_d[0:full, :].rearrange("(t p) j -> p t j", p=128),
                out_stage[:, 0:full // 128, :])
        if NSH > full:
            nc.sync.dma_start(out_d[full:NSH, :],
                              out_stage[0:NSH - full, NT - 1, :])

    nc.compile()
    return nc


def _wr_rows(nc, dst, r0, rows, st, width, col0=0):
    """DMA staging [128, G, width] (rows r = g*128+p at [p, g]) to DRAM rows
    dst[r0:r0+rows, 0:width] (dst rows may be wider than the payload)."""
    g_full = rows // 128
    if g_full:
        nc.sync.dma_start(
            dst[r0:r0 + g_full * 128, 0:width]
            .rearrange("(g p) c -> p g c", p=128),
            st[:, col0:col0 + g_full, :])
    rem = rows - g_full * 128
    if rem:
        nc.sync.dma_start(dst[r0 + g_full * 128:r0 + rows, 0:width],
                          st[0:rem, col0 + g_full, :])


# ----------------------------------------------------------------------------
# entry point
# ----------------------------------------------------------------------------

_CACHE = {}


def kernel(x, edge_index, edge_attr, W1, att_src1, att_dst1, b1,
           W2, att_src2, att_dst2, b2, ff1_w, ff1_b, ff2_w, ff2_b):
    x = np.asarray(x, np.float32)
    edge_index = np.asarray(edge_index)
    args = [np.asarray(a, np.float32) for a in
            (W1, att_src1, att_dst1, b1, W2, att_src2, att_dst2, b2,
             ff1_w, ff1_b, ff2_w, ff2_b)]
    in_maps, sched, dims = _prep(x, edge_index, *args)
    key = (dims["N"], dims["IN"], tuple(sched.n_lo), tuple(sched.n_hi))
    if key not in _CACHE:
        _CACHE[key] = _build(sched, dims)
    nc = _CACHE[key]
    salt = np.zeros((1, 4), np.float32)
    for m in in_maps:
        m[nc._salt_name] = salt
    res = run_bass_kernel_spmd(nc, in_maps, list(range(NCORES))).results
    out = np.concatenate([res[k]["out"] for k in range(NCORES)], axis=0)
    return out.astype(np.float32)



# revision 69
# speedup vs baseline: 1.0078x; 1.0064x over previous
"""GAT (2-layer GATConv + FF head) on 8 Trainium2 NeuronCores.

Strategy (per sharding hint): nodes + incident edges partitioned by
destination across 8 cores; per-edge softmax/scatter local to the
destination shard via one-hot matmul-scatter into PSUM; small weights
replicated.

v3 layout: both layers are symmetric [sharded dense -> AllGather h-table ->
edge phase]. The dense transform uses an augmented weight [W | v_s | v_d] so
each node row carries [h | a_src | a_dst] in one matmul; [h | a_src] goes to
the gather table (768B-stride rows: SWDGE rows must be 256B multiples, so
260 payload cols pad to 384), a_dst to a local [NT,H] stash. The per-edge
gather then delivers a_src for free -- no per-edge DVE mult+reduce.

Edge phase per tile-group: gather rows, build the one-hot on DVE (packed-
pair views keep every operand 2-byte/packed-last-dim -> DVE 2x mode), get
the transposed one-hot via PE transpose + Act PSUM->SBUF copy, a_dst via a
one-hot matmul, alpha/lrelu on DVE, exp on Act straight into the row's
a_src slot so the scatter is ONE 260-col matmul per chunk (a single PSUM
accumulation group -- interleaved groups mis-accumulate on HW).

The emission is software-pipelined FRONT(i)/BACK(i-1)/EVICT(i-2): engine
queues execute in-order, so without it alpha's psA wait and evict's PSUM
read head-of-line-block the next group's independent work (~60% of the
edge-phase time).

Message path in fp16 (tables, gathered rows, attention weights, matmul
operands); accumulation in fp32 PSUM; per-edge logits in fp32.
"""
import sys
sys.path.insert(0, "/opt/trn_rl_repo")

import numpy as np
from contextlib import ExitStack

import concourse.bass as bass
import concourse.bacc as bacc
import concourse.tile as tile
import concourse.mybir as mybir
from concourse.bass_utils import run_bass_kernel_spmd

dt = mybir.dt
OP = mybir.AluOpType
ACT = mybir.ActivationFunctionType

NCORES = 8
H = 4
NEG_SLOPE = 0.2


# ----------------------------------------------------------------------------
# host-side prep
# ----------------------------------------------------------------------------

def _wrap_idx(idx):
    """Pack an index list into the SWDGE wrapped layout [128, n/16] int16:
    index i -> partition i%16 (replicated to all 8 16-partition groups),
    free offset i//16."""
    n = len(idx)
    assert n % 128 == 0
    out = np.zeros((128, n // 16), np.int16)
    a = np.asarray(idx, np.int16).reshape(n // 16, 16).T  # [16, n/16]
    for r in range(8):
        out[r * 16:(r + 1) * 16, :] = a
    return out


def _pad128(a, fill):
    n = len(a)
    m = ((n + 127) // 128) * 128
    return np.concatenate([a, np.full(m - n, fill, a.dtype)])


class Sched:
    """Static, core-uniform per-tile chunk schedule."""

    def __init__(self, n_lo, n_hi):
        self.n_lo = n_lo          # [NT] chunks for lo-half gathers
        self.n_hi = n_hi          # [NT] chunks for hi-half gathers
        self.ct = [a + b for a, b in zip(n_lo, n_hi)]
        self.base = np.concatenate([[0], np.cumsum(self.ct)]).astype(int)
        self.total = int(self.base[-1])  # total chunks per core


def _prep(x, edge_index, W1, att_src1, att_dst1, b1, W2, att_src2, att_dst2,
          b2, ff1_w, ff1_b, ff2_w, ff2_b):
    N, IN = x.shape
    F = W1.shape[1]               # 256
    C1 = F // H
    C2 = W2.shape[1] // H
    NSH = N // NCORES
    NT = (NSH + 127) // 128
    # lo/hi split is per-shard (local rows [0:NSH_LO) of each core's shard
    # go to the lo table) so each half can be AllGathered separately and
    # overlap with the other half's gathers. 128-aligned; both halves'
    # table sizes stay within int16 gather indexing.
    NSH_LO = ((NSH // 2) // 128) * 128
    NSH_HI = NSH - NSH_LO

    E = edge_index.shape[1]
    ar = np.arange(N, dtype=np.int64)
    src = np.concatenate([edge_index[0], ar])
    dst = np.concatenate([edge_index[1], ar])

    shard = dst // NSH
    dstloc_all = dst - shard * NSH

    # group edges per (core, tile, half)
    per = [[[None, None] for _ in range(NT)] for _ in range(NCORES)]
    for k in range(NCORES):
        m = shard == k
        s_k, dl_k = src[m], dstloc_all[m]
        t_k = dl_k // 128
        for t in range(NT):
            mt = t_k == t
            s_t, dl_t = s_k[mt], dl_k[mt]
            s_shard, s_loc = s_t // NSH, s_t % NSH
            lo = s_loc < NSH_LO
            per[k][t][0] = (s_shard[lo] * NSH_LO + s_loc[lo], dl_t[lo])
            per[k][t][1] = (s_shard[~lo] * NSH_HI + (s_loc[~lo] - NSH_LO),
                            dl_t[~lo])

    n_lo = [max((len(per[k][t][0][0]) + 127) // 128 for k in range(NCORES))
            for t in range(NT)]
    n_hi = [max((len(per[k][t][1][0]) + 127) // 128 for k in range(NCORES))
            for t in range(NT)]
    sched = Sched(n_lo, n_hi)

    # Per-group chunk layout: [t0-lo .. | t0-hi ..] so each half's gather
    # region is contiguous across the group (fewer dma_gather calls).
    # sched.order[j] = (t, half) of the j-th chunk group in schedule order.
    import os
    PS = int(os.environ.get('K_PS', '1'))
    order = []           # flat chunk -> (t, half)
    pair_lo = []         # per group: (#lo chunks, #hi chunks)
    for p0 in range(0, NT, PS):
        ts = [t for t in range(p0, min(p0 + PS, NT))]
        nlo = sum(n_lo[t] for t in ts)
        nhi = sum(n_hi[t] for t in ts)
        pair_lo.append((nlo, nhi))
        for t in ts:
            order += [(t, 0)] * n_lo[t]
        for t in ts:
            order += [(t, 1)] * n_hi[t]
    sched.order = order
    sched.pair_lo = pair_lo
    sched.ps = PS
    # chunk offset of (t, half) within the flat schedule
    choff = {}
    off = 0
    for (t, half) in order:
        choff.setdefault((t, half), off)
        off += 1
    sched.choff = choff

    # per-core edge arrays in schedule order
    idx_h = []       # [128, total*8] int16  (gather idx, lo/hi-local rows)
    dstloc = []      # [128, total] f16      (tile-local dst or -1, p=edge%128)
    for k in range(NCORES):
        ih = np.zeros((128, sched.total * 8), np.int16)
        dl = np.full((128, sched.total), -1.0, np.float16)
        for t in range(NT):
            for half, nch in ((0, n_lo[t]), (1, n_hi[t])):
                if nch == 0:
                    continue
                off = choff[(t, half)]
                s_t, dl_t = per[k][t][half]
                ne = nch * 128
                sp = _pad128(np.concatenate([s_t, np.zeros(ne - len(s_t),
                                                           np.int64)]), 0)[:ne]
                sp[len(s_t):] = 0
                dlp = np.full(ne, -1.0, np.float32)
                dlp[:len(dl_t)] = (dl_t - t * 128).astype(np.float32)
                ih[:, off * 8:(off + nch) * 8] = _wrap_idx(sp)
                dl[:, off:off + nch] = dlp.reshape(nch, 128).T.astype(np.float16)
        idx_h.append(ih)
        dstloc.append(dl)

    # weights: [W | v_s | v_d]. The dense phase emits [h | a_src | a_dst];
    # [h | a_src] goes to the gather table row, a_dst to the local stash.
    def aug(W, a_s, a_d, C):
        v_s = np.einsum("fhc,hc->fh", W.reshape(-1, H, C), a_s)
        v_d = np.einsum("fhc,hc->fh", W.reshape(-1, H, C), a_d)
        return np.concatenate([W, v_s, v_d], axis=1).astype(np.float16)

    W1aug = aug(W1, att_src1, att_dst1, C1)              # [IN, F+8]
    W2aug = aug(W2, att_src2, att_dst2, C2)              # [F, F+8]
    W2aug_pk = W2aug.reshape(2, 128, F + 8).transpose(1, 0, 2).copy()

    xT16 = np.ascontiguousarray(x.T).astype(np.float16)  # [IN, N]

    iota16 = np.tile(np.arange(128, dtype=np.float16), (128, 1))
    ident16 = np.eye(128, dtype=np.float16)

    const = {
        "W1aug": W1aug, "W2aug": W2aug_pk,
        "iota16": iota16, "ident16": ident16,
        "b1rep": np.tile(b1.astype(np.float32), (128, 1)),
        "b2rep": np.tile(b2.astype(np.float32), (128, 1)),
        "f1brep": np.tile(ff1_b.astype(np.float32), (128, 1)),
        "f2brep": np.tile(ff2_b.astype(np.float32), (128, 1)),
        "ff1w16": ff1_w.astype(np.float16),
        "ff2w16": ff2_w.astype(np.float16),
    }

    in_maps = []
    for k in range(NCORES):
        m = dict(const)
        m.update({
            "xT16": np.ascontiguousarray(xT16[:, k * NSH:(k + 1) * NSH]),
            "idx_h": idx_h[k], "dstloc": dstloc[k],
        })
        in_maps.append(m)

    dims = dict(N=N, IN=IN, F=F, NSH=NSH, NT=NT, NSH_LO=NSH_LO,
                NSH_HI=NSH_HI, C2=C2, FH=ff1_w.shape[1])
    return in_maps, sched, dims


# ----------------------------------------------------------------------------
# device program
# ----------------------------------------------------------------------------

def _gather_split(nc, out_ap_fn, tab, idx_sb, n_chunks, elem, q0):
    """Emit dma_gather calls capped at 8 chunks (1024 idxs) each.
    out_ap_fn(c0, c1) -> output AP for chunk range; idx_sb indexed per chunk.
    single_packet=False for rows >=512B (measured 286 vs 55 GB/s); True is
    faster for 256B rows (145-173 GB/s)."""
    sp = elem * 2 < 512
    c0 = 0
    q = q0
    while c0 < n_chunks:
        c1 = min(c0 + 8, n_chunks)
        nc.gpsimd.dma_gather(
            out_ap_fn(c0, c1), tab, idx_sb[:, c0 * 8:c1 * 8],
            num_idxs=(c1 - c0) * 128, num_idxs_reg=(c1 - c0) * 128,
            elem_size=elem, queue_num=q % 4, single_packet=sp)
        q += 1
        c0 = c1


def _build(sched, dims):
    import os, hashlib
    PH = int(os.environ.get('K_PHASES', '6'))
    TAPS = int(os.environ.get('K_TAPS', '0'))
    SIM = int(os.environ.get('K_SIM', '0'))
    PS = sched.ps
    BUFS = int(os.environ.get('K_BUFS', '6'))
    DLAG = int(os.environ.get('K_D', '2'))
    # phase repeat counts (profiling aid: phase time = dE/(R-1))
    R1 = int(os.environ.get('K_R1', '1'))
    R2 = int(os.environ.get('K_R2', '1'))
    R3 = int(os.environ.get('K_R3', '1'))
    R4 = int(os.environ.get('K_R4', '1'))
    R5 = int(os.environ.get('K_R5', '1'))
    R6 = int(os.environ.get('K_R6', '1'))
    N, IN, F, NSH, NT = dims["N"], dims["IN"], dims["F"], dims["NSH"], dims["NT"]
    NSH_LO, NSH_HI = dims["NSH_LO"], dims["NSH_HI"]
    NLO_T, NHI_T = NSH_LO * NCORES, NSH_HI * NCORES
    C2, FH = dims["C2"], dims["FH"]
    FA = F + 8       # dense output: [h | a_src | a_dst]
    FT = F + 4       # table row payload: [h | a_src]
    FR = 384         # table row stride (SWDGE needs 256B-multiple rows)

    nc = bacc.Bacc("TRN2", target_bir_lowering=False,
                   num_devices=(1 if SIM else NCORES), num_swdge_queues=4)
    # The neuronx compile cache keys on the jit signature only (the embedded
    # BIR is not hashed), so two different programs with identical I/O would
    # alias to one NEFF. A content-named dummy input de-aliases them.
    with open(__file__, "rb") as _f:
        _salt = hashlib.sha256(
            _f.read() + repr((sched.n_lo, sched.n_hi, sched.order,
                              sorted(dims.items()),
                              PH, BUFS, DLAG, R1, R2, R3, R4, R5, R6)).encode()
        ).hexdigest()[:16]
    nc.dram_tensor(f"salt_{_salt}", [1, 4], dt.float32, kind="ExternalInput")
    dims["salt_name"] = f"salt_{_salt}"
    nc._salt_name = f"salt_{_salt}"

    # inputs
    xT16 = nc.dram_tensor("xT16", [IN, NSH], dt.float16, kind="ExternalInput")
    W1aug = nc.dram_tensor("W1aug", [IN, FA], dt.float16, kind="ExternalInput")
    W2aug = nc.dram_tensor("W2aug", [128, 2, FA], dt.float16, kind="ExternalInput")
    iota_in = nc.dram_tensor("iota16", [128, 128], dt.float16, kind="ExternalInput")
    ident_in = nc.dram_tensor("ident16", [128, 128], dt.float16, kind="ExternalInput")
    b1rep = nc.dram_tensor("b1rep", [128, F], dt.float32, kind="ExternalInput")
    b2rep = nc.dram_tensor("b2rep", [128, C2], dt.float32, kind="ExternalInput")
    f1brep = nc.dram_tensor("f1brep", [128, FH], dt.float32, kind="ExternalInput")
    f2brep = nc.dram_tensor("f2brep", [128, 2], dt.float32, kind="ExternalInput")
    ff1w16 = nc.dram_tensor("ff1w16", [C2, FH], dt.float16, kind="ExternalInput")
    ff2w16 = nc.dram_tensor("ff2w16", [FH, 2], dt.float16, kind="ExternalInput")
    idx_h_d = nc.dram_tensor("idx_h", [128, sched.total * 8], dt.int16, kind="ExternalInput")
    dstloc_d = nc.dram_tensor("dstloc", [128, sched.total], dt.float16, kind="ExternalInput")

    out_d = nc.dram_tensor("out", [NSH, 2], dt.float32, kind="ExternalOutput")
    if TAPS:
        tap_t1 = nc.dram_tensor("tap_t1", [256, FT], dt.float16, kind="ExternalOutput")
        tap_ag1 = nc.dram_tensor("tap_ag1", [256, FT], dt.float16, kind="ExternalOutput")
        tap_h1 = nc.dram_tensor("tap_h1", [NSH, F], dt.float16, kind="ExternalOutput")
        tap_ag2 = nc.dram_tensor("tap_ag2", [256, FT], dt.float16, kind="ExternalOutput")
        tap_h2 = nc.dram_tensor("tap_h2", [NSH, C2], dt.float16, kind="ExternalOutput")

    with tile.TileContext(nc) as tc, ExitStack() as octx:
        # persistent pools
        dram = octx.enter_context(tc.tile_pool(name="dram", bufs=1, space="DRAM"))
        cpool = octx.enter_context(tc.tile_pool(name="const", bufs=1))
        stash = octx.enter_context(tc.tile_pool(name="stash", bufs=1))

        # DRAM tables ([h | a_src | 124-col pad] rows, 768B stride).
        # Each layer's table is exchanged as TWO per-shard-half AllGathers
        # (lo = local rows [0:NSH_LO) of every shard) so the lo-half edge
        # gathers can start while the hi half is still in flight.
        t1h_own = dram.tile([NSH, FR], dt.float16)
        t2h_own = dram.tile([NSH, FR], dt.float16)
        _aspace = {} if SIM else {"addr_space": "Shared"}
        t1lo = dram.tile([NLO_T, FR], dt.float16, **_aspace)
        t1hi = dram.tile([NHI_T, FR], dt.float16, **_aspace)
        t2lo = dram.tile([NLO_T, FR], dt.float16, **_aspace)
        t2hi = dram.tile([NHI_T, FR], dt.float16, **_aspace)

        # constants in SBUF
        iota16 = cpool.tile([128, 128], dt.float16)
        nc.sync.dma_start(iota16[:], iota_in[:])
        ident16 = cpool.tile([128, 128], dt.float16)
        nc.sync.dma_start(ident16[:], ident_in[:])
        w1a_sb = cpool.tile([IN, FA], dt.float16)
        nc.sync.dma_start(w1a_sb[:], W1aug[:])
        w2a_sb = cpool.tile([128, 2, FA], dt.float16)
        nc.sync.dma_start(w2a_sb[:], W2aug[:])
        b1_sb = cpool.tile([128, F], dt.float32)
        nc.sync.dma_start(b1_sb[:], b1rep[:])
        b2_sb = cpool.tile([128, C2], dt.float32)
        nc.sync.dma_start(b2_sb[:], b2rep[:])
        f1b_sb = cpool.tile([128, FH], dt.float32)
        nc.sync.dma_start(f1b_sb[:], f1brep[:])
        f2b_sb = cpool.tile([128, 2], dt.float32)
        nc.sync.dma_start(f2b_sb[:], f2brep[:])
        ff1_sb = cpool.tile([C2, FH], dt.float16)
        nc.sync.dma_start(ff1_sb[:], ff1w16[:])
        ff2_sb = cpool.tile([FH, 2], dt.float16)
        nc.sync.dma_start(ff2_sb[:], ff2w16[:])

        # layer-1 hidden transposed, kept in SBUF for the layer-2 dense
        h1T = stash.tile([128, 2, NT, 128], dt.float16)
        out_stage = stash.tile([128, NT, 2], dt.float32)
        # per-own-tile a_dst projections, SBUF-resident.
        # Zeroed first: the a_dst matmul contracts over all 128 partitions and
        # 0 * uninitialized-NaN would poison PSUM (partial last tile).
        n1stash = stash.tile([128, NT, H], dt.float16)
        nc.vector.memset(n1stash[:], 0.0)
        n2stash = stash.tile([128, NT, H], dt.float16)
        nc.vector.memset(n2stash[:], 0.0)

        # ------------------------------------------------------------------
        # phase A: sharded layer-1 dense -> t1h_own / t1n_own
        # ------------------------------------------------------------------
        for _rep1 in range(R1):
         with ExitStack() as ctx:
            xp = ctx.enter_context(tc.tile_pool(name="xp", bufs=2))
            pp = ctx.enter_context(tc.tile_pool(name="pp", bufs=4, space="PSUM"))
            sp = ctx.enter_context(tc.tile_pool(name="sp", bufs=2))

            G = 8
            t0 = 0
            while t0 < NT:
                g = min(G, NT - t0)
                rows_g = min(g * 128, NSH - t0 * 128)
                xs = xp.tile([IN, G * 128], dt.float16, tag="xs")
                nc.sync.dma_start(xs[:, 0:rows_g],
                                  xT16[:, t0 * 128:t0 * 128 + rows_g])
                hst = sp.tile([128, G, FT], dt.float16, tag="hst")
                for j in range(g):
                    t = t0 + j
                    rows = min(128, NSH - t * 128)
                    ps = pp.tile([128, FA], dt.float32, tag="ps")
                    nc.tensor.matmul(ps[0:rows, :], xs[:, j * 128:j * 128 + rows],
                                     w1a_sb[:], start=True, stop=True)
                    nc.scalar.activation(hst[0:rows, j, :], ps[0:rows, 0:FT],
                                         ACT.Copy)
                    nc.vector.tensor_copy(n1stash[0:rows, t, :],
                                          ps[0:rows, FT:FA])
                _wr_rows(nc, t1h_own, t0 * 128, rows_g, hst, FT)
                t0 += g

        # ------------------------------------------------------------------
        # exchange 1
        # ------------------------------------------------------------------
        if PH >= 2:
          for _rep2 in range(R2):
            if SIM:
                for k in range(NCORES):
                    nc.sync.dma_start(
                        t1lo[k * NSH_LO:(k + 1) * NSH_LO, :],
                        t1h_own[0:NSH_LO, :])
                for k in range(NCORES):
                    nc.sync.dma_start(
                        t1hi[k * NSH_HI:(k + 1) * NSH_HI, :],
                        t1h_own[NSH_LO:NSH, :])
            else:
                tl, th = t1lo, t1hi
                if _rep2 != R2 - 1:
                    tl = dram.tile([NLO_T, FR], dt.float16,
                                   addr_space="Shared", name=f"t1lr{_rep2}")
                    th = dram.tile([NHI_T, FR], dt.float16,
                                   addr_space="Shared", name=f"t1hr{_rep2}")
                nc.gpsimd.collective_compute(
                    "AllGather", OP.bypass,
                    replica_groups=[list(range(NCORES))],
                    ins=[t1h_own[0:NSH_LO, :].opt()], outs=[tl[:].opt()])
                nc.gpsimd.collective_compute(
                    "AllGather", OP.bypass,
                    replica_groups=[list(range(NCORES))],
                    ins=[t1h_own[NSH_LO:NSH, :].opt()], outs=[th[:].opt()])

        # ------------------------------------------------------------------
        # edge phases
        # ------------------------------------------------------------------
        def edge_phase(ctx, name, tab_lo, tab_hi, nstash, evict):
            # Tiles are processed in PAIRS: all elementwise stages run batched
            # over the pair's chunk range. Within a pair, chunks are ordered
            # [t0-lo | t1-lo | t0-hi | t1-hi] so each half's gather region is
            # contiguous. The transposed one-hot (for the a_dst matmul) is
            # built by PE-transposing oh; Activation evacuates PSUM->SBUF.
            # Software pipeline: FRONT(i) [loads, gathers, oh, ohT, psA] /
            # BACK(i-1) [alpha..fold, scatter] / EVICT(i-2). Engine queues
            # are in-order; without this, evict's PSUM read and alpha's psA
            # wait head-of-line-block the next group's independent work.
            ep = ctx.enter_context(tc.tile_pool(name=name + "e", bufs=BUFS))
            pp = ctx.enter_context(tc.tile_pool(name=name + "p", bufs=2, space="PSUM"))
            groups = []
            for ip, p0 in enumerate(range(0, NT, PS)):
                p1 = min(p0 + PS, NT)
                tiles = [t for t in range(p0, p1) if sched.ct[t] > 0]
                if not tiles:
                    continue
                b0 = sched.base[p0]
                gct = sched.base[p1] - b0
                ctile = [sched.order[b0 + j][0] for j in range(gct)]
                groups.append(dict(
                    ip=ip, b0=b0, gct=gct, tiles=tiles, ctile=ctile,
                    nlo=sched.pair_lo[ip][0], nhi=sched.pair_lo[ip][1],
                    first={t: min(j for j in range(gct) if ctile[j] == t)
                           for t in tiles},
                    last={t: max(j for j in range(gct) if ctile[j] == t)
                          for t in tiles}))

            def front(G):
                b0, gct = G["b0"], G["gct"]
                ixh = ep.tile([128, gct * 8], dt.int16, tag="ixh")
                nc.sync.dma_start(ixh[:], idx_h_d[:, b0 * 8:(b0 + gct) * 8])
                dl = ep.tile([128, gct], dt.float16, tag="dl")
                nc.sync.dma_start(dl[:], dstloc_d[:, b0:b0 + gct])

                g = G["g"] = ep.tile([128, gct, FR], dt.float16, tag="g",
                                     name="g")
                if G["nlo"]:
                    _gather_split(
                        nc, lambda a, b: g[:, a:b, :], tab_lo,
                        ixh[:, 0:G["nlo"] * 8], G["nlo"], FR, 0)
                if G["nhi"]:
                    _gather_split(
                        nc, lambda a, b, o=G["nlo"]: g[:, o + a:o + b, :],
                        tab_hi, ixh[:, G["nlo"] * 8:gct * 8], G["nhi"], FR, 2)

                # one-hot: oh[e, c, d] = (iota[e, d] == dl[e, c]).
                # dl2 pre-broadcast (last-dim pair) keeps every operand's
                # last dim packed 2-byte -> DVE 2x mode.
                dl2 = ep.tile([128, gct, 2], dt.float16, tag="dl2")
                nc.vector.tensor_copy(
                    dl2[:], dl[:].unsqueeze(2).broadcast_to([128, gct, 2]))
                oh = G["oh"] = ep.tile([128, gct, 128], dt.float16, tag="oh",
                                       name="oh")
                nc.vector.tensor_tensor(
                    oh[:].rearrange("p c (d e) -> p c d e", e=2),
                    iota16[:].rearrange("p (d e) -> p d e", e=2)
                        .unsqueeze(1).broadcast_to([128, gct, 64, 2]),
                    dl2[:].unsqueeze(2).broadcast_to([128, gct, 64, 2]),
                    op=OP.is_equal)

                # ohT via PE transpose (4 chunks per PSUM tile; Act evacuates)
                ohT = ep.tile([128, gct, 128], dt.float16, tag="ohT")
                for j0 in range(0, gct, 8):
                    j1 = min(j0 + 8, gct)
                    tp4 = pp.tile([128, 8, 128], dt.float16, tag="tp4", bufs=1)
                    for j in range(j0, j1):
                        nc.tensor.transpose(tp4[:, j - j0, :], oh[:, j, :],
                                            ident16[:])
                    nc.scalar.activation(ohT[:, j0:j1, :], tp4[:, 0:j1 - j0, :],
                                         ACT.Copy)

                # a_dst per edge: psA[e, (c)H+h] = nstash[dst_e, t(c), h]
                psA = G["psA"] = pp.tile([128, gct * H], dt.float32,
                                         tag="psA", bufs=DLAG + 1,
                                         name="psA")
                for j in range(gct):
                    nc.tensor.matmul(
                        psA[:, j * H:(j + 1) * H],
                        ohT[:, j, :], nstash[:, G["ctile"][j], :],
                        start=True, stop=True)

            def back(G):
                gct, g, psA = G["gct"], G["g"], G["psA"]
                # alpha = lrelu(a_src + a_dst); a_src rides in the gathered
                # row (cols F:F+H). ex = exp(alpha)
                alpha = ep.tile([128, gct, H], dt.float32, tag="alpha")
                nc.vector.tensor_tensor(
                    alpha[:], g[:, :, F:F + H],
                    psA[:].rearrange("p (c h) -> p c h", h=H),
                    op=OP.add)
                nc.vector.scalar_tensor_tensor(
                    alpha[:], alpha[:], float(NEG_SLOPE), alpha[:],
                    op0=OP.mult, op1=OP.max)
                # exp lands in g's a_src slot (already consumed by alpha) so
                # the scatter is ONE 260-col matmul per chunk -> a single
                # PSUM accumulation group (interleaved groups mis-accumulate)
                nc.scalar.activation(g[:, :, F:F + H], alpha[:], ACT.Exp)
                # fold in place: g *= ex (per-head broadcast); ex2 pre-pairs
                # the broadcast so all last dims stay packed -> DVE 2x.
                ex2 = ep.tile([128, gct, H, 2], dt.float16, tag="ex2")
                nc.vector.tensor_copy(
                    ex2[:], g[:, :, F:F + H].unsqueeze(3)
                    .broadcast_to([128, gct, H, 2]))
                nc.vector.tensor_tensor(
                    g[:, :, 0:F].rearrange("p c (h d e) -> p c h d e",
                                           h=H, e=2),
                    g[:, :, 0:F].rearrange("p c (h d e) -> p c h d e",
                                           h=H, e=2),
                    ex2[:].unsqueeze(3).broadcast_to(
                        [128, gct, H, F // H // 2, 2]),
                    op=OP.mult)
                # matmul-scatter per chunk: [messages | denominators] in one
                # accumulation group per tile
                G["pss"] = {}
                for t in G["tiles"]:
                    G["pss"][t] = pp.tile([128, F + H], dt.float32, tag="ps",
                                          bufs=DLAG + 1, name=f"ps_t{t}")
                for j in range(gct):
                    t = G["ctile"][j]
                    nc.tensor.matmul(
                        G["pss"][t][:, 0:F + H], G["oh"][:, j, :],
                        g[:, j, 0:F + H],
                        start=(j == G["first"][t]), stop=(j == G["last"][t]))

            def ev(G):
                for t in G["tiles"]:
                    evict(ep, pp, t, G["pss"][t])

            ng = len(groups)
            for i in range(ng + 2 * DLAG):
                if i < ng:
                    front(groups[i])
                if 0 <= i - DLAG < ng:
                    back(groups[i - DLAG])
                if 0 <= i - 2 * DLAG < ng:
                    ev(groups[i - 2 * DLAG])


        # ---- layer 1 evict: h1 = relu(agg/den + b1); build h1T ----
        def evict1(ep, pp, t, ps):
            rows = min(128, NSH - t * 128)
            rcp = ep.tile([128, H], dt.float32, tag="rcp")
            nc.vector.reciprocal(rcp[:], ps[:, F:F + H])
            pre = ep.tile([128, F], dt.float32, tag="pre")
            nc.vector.tensor_tensor(
                pre[:].rearrange("p (h d) -> p h d", h=H),
                ps[:, 0:F].rearrange("p (h d) -> p h d", h=H),
                rcp[:].unsqueeze(2).broadcast_to([128, H, F // H]), op=OP.mult)
            nc.vector.tensor_tensor(pre[:], pre[:], b1_sb[:], op=OP.add)
            h1r = ep.tile([128, F], dt.float16, tag="h1r")
            nc.scalar.activation(h1r[:], pre[:], ACT.Relu)
            if TAPS:
                nc.sync.dma_start(tap_h1[t * 128:t * 128 + rows, :],
                                  h1r[0:rows, :])
            tp = pp.tile([128, 4, 128], dt.float16, tag="tpe", bufs=1,
                         name="tp_ev1")
            for b in range(2):
                nc.tensor.transpose(tp[:, b, :], h1r[:, b * 128:(b + 1) * 128],
                                    ident16[:])
            nc.scalar.activation(h1T[:, :, t, :], tp[:, 0:2, :], ACT.Copy)

        if PH >= 3:
         for _rep3 in range(R3):
          with ExitStack() as ctx:
            edge_phase(ctx, "l1", t1lo[:], t1hi[:], n1stash, evict1)

        # ------------------------------------------------------------------
        # phase C: layer-2 dense on own rows -> t2h_own / t2n_own
        # ------------------------------------------------------------------
        if PH >= 4:
         for _rep4 in range(R4):
          with ExitStack() as ctx:
            cp = ctx.enter_context(tc.tile_pool(name="cp", bufs=2))
            pp = ctx.enter_context(tc.tile_pool(name="cpp", bufs=4, space="PSUM"))
            G = 8
            t0 = 0
            while t0 < NT:
                g = min(G, NT - t0)
                hst = cp.tile([128, G, FT], dt.float16, tag="hst")
                for j in range(g):
                    t = t0 + j
                    rows = min(128, NSH - t * 128)
                    ps = pp.tile([128, FA], dt.float32, tag="ps")
                    for b in range(2):
                        nc.tensor.matmul(ps[:], h1T[:, b, t, :], w2a_sb[:, b, :],
                                         start=(b == 0), stop=(b == 1))
                    nc.scalar.activation(hst[:, j, :], ps[:, 0:FT], ACT.Copy)
                    nc.vector.tensor_copy(n2stash[0:rows, t, :],
                                          ps[0:rows, FT:FA])
                rows_t = min(g * 128, NSH - t0 * 128)
                _wr_rows(nc, t2h_own, t0 * 128, rows_t, hst, FT)
                t0 += g

        # ------------------------------------------------------------------
        # exchange 2
        # ------------------------------------------------------------------
        if PH >= 5:
          for _rep5 in range(R5):
            if SIM:
                for k in range(NCORES):
                    nc.sync.dma_start(
                        t2lo[k * NSH_LO:(k + 1) * NSH_LO, :],
                        t2h_own[0:NSH_LO, :])
                for k in range(NCORES):
                    nc.sync.dma_start(
                        t2hi[k * NSH_HI:(k + 1) * NSH_HI, :],
                        t2h_own[NSH_LO:NSH, :])
            else:
                tl, th = t2lo, t2hi
                if _rep5 != R5 - 1:
                    tl = dram.tile([NLO_T, FR], dt.float16,
                                   addr_space="Shared", name=f"t2lr{_rep5}")
                    th = dram.tile([NHI_T, FR], dt.float16,
                                   addr_space="Shared", name=f"t2hr{_rep5}")
                nc.gpsimd.collective_compute(
                    "AllGather", OP.bypass,
                    replica_groups=[list(range(NCORES))],
                    ins=[t2h_own[0:NSH_LO, :].opt()], outs=[tl[:].opt()])
                nc.gpsimd.collective_compute(
                    "AllGather", OP.bypass,
                    replica_groups=[list(range(NCORES))],
                    ins=[t2h_own[NSH_LO:NSH, :].opt()], outs=[th[:].opt()])

        # ---- layer 2 evict: h2 = relu(mean_h(agg/den) + b2); FF head ----
        def evict2(ep, pp, t, ps):
            rows = min(128, NSH - t * 128)
            rcp = ep.tile([128, H], dt.float32, tag="rcp")
            nc.vector.reciprocal(rcp[:], ps[:, F:F + H])
            pre = ep.tile([128, H, C2], dt.float32, tag="pre")
            nc.vector.tensor_tensor(
                pre[:], ps[:, 0:F].rearrange("p (h d) -> p h d", h=H),
                rcp[:].unsqueeze(2).broadcast_to([128, H, C2]), op=OP.mult)
            red = ep.tile([128, C2], dt.float32, tag="red")
            nc.vector.tensor_reduce(red[:], pre[:].transpose([0, 2, 1]),
                                    axis=mybir.AxisListType.X, op=OP.add)
            nc.vector.scalar_tensor_tensor(red[:], red[:], 1.0 / H, b2_sb[:],
                                           op0=OP.mult, op1=OP.add)
            h2 = ep.tile([128, 128], dt.float16, tag="h2")
            nc.vector.memset(h2[:, C2:128], 0.0)
            nc.scalar.activation(h2[:, 0:C2], red[:], ACT.Relu)
            if TAPS:
                nc.sync.dma_start(tap_h2[t * 128:t * 128 + rows, :],
                                  h2[0:rows, 0:C2])
            # FF: out = relu(h2 @ ff1 + b1f) @ ff2 + b2f  (square transposes)
            tp = pp.tile([128, 4, 128], dt.float16, tag="tpe", bufs=1,
                         name="tp_ev2a")
            nc.tensor.transpose(tp[:, 0, :], h2[:], ident16[:])
            h2T = ep.tile([C2, 128], dt.float16, tag="h2T")
            nc.scalar.activation(h2T[:], tp[0:C2, 0, :], ACT.Copy)
            pf1 = pp.tile([128, FH], dt.float32, tag="tpe", bufs=1,
                          name="pf1")
            nc.tensor.matmul(pf1[:], h2T[:], ff1_sb[:], start=True, stop=True)
            f1p = ep.tile([128, FH], dt.float32, tag="f1p")
            nc.vector.tensor_tensor(f1p[:], pf1[:], f1b_sb[:], op=OP.add)
            f1 = ep.tile([128, 128], dt.float16, tag="f1")
            nc.vector.memset(f1[:, FH:128], 0.0)
            nc.scalar.activation(f1[:, 0:FH], f1p[:], ACT.Relu)
            tpf = pp.tile([128, 4, 128], dt.float16, tag="tpe", bufs=1,
                          name="tp_ev2b")
            nc.tensor.transpose(tpf[:, 0, :], f1[:], ident16[:])
            f1T = ep.tile([FH, 128], dt.float16, tag="f1T")
            nc.scalar.activation(f1T[:], tpf[0:FH, 0, :], ACT.Copy)
            pf2 = pp.tile([128, 2], dt.float32, tag="tpe", bufs=1,
                          name="pf2")
            nc.tensor.matmul(pf2[:], f1T[:], ff2_sb[:], start=True, stop=True)
            nc.vector.tensor_tensor(out_stage[:, t, :], pf2[:], f2b_sb[:],
                                    op=OP.add)

        if TAPS:
            nc.sync.dma_start(tap_t1[:], t1h_own[0:256, 0:FT])
            if PH >= 2:
                nc.sync.dma_start(tap_ag1[:], t1lo[NSH_LO:NSH_LO + 256, 0:FT])
            if PH >= 5:
                nc.sync.dma_start(tap_ag2[:], t2lo[NSH_LO:NSH_LO + 256, 0:FT])
        if PH >= 6:
         for _rep6 in range(R6):
          with ExitStack() as ctx:
            edge_phase(ctx, "l2", t2lo[:], t2hi[:], n2stash, evict2)

        # final output
        if PH < 6:
            nc.vector.memset(out_stage[:], 0.0)
        full = (NSH // 128) * 128
        if full:
            nc.sync.dma_start(
                out_d[0:full, :].rearrange("(t p) j -> p t j", p=128),
                out_stage[:, 0:full // 128, :])
        if NSH > full:
            nc.sync.dma_start(out_d[full:NSH, :],
                              out_stage[0:NSH - full, NT - 1, :])

    nc.compile()
    return nc


def _wr_rows(nc, dst, r0, rows, st, width, col0=0):
    """DMA staging [128, G, width] (rows r = g*128+p at [p, g]) to DRAM rows
    dst[r0:r0+rows, 0:width] (dst rows may be wider than the payload)."""
    g_full = rows // 128
    if g_full:
        nc.sync.dma_start(
            dst[r0:r0 + g_full * 128, 0:width]
            .rearrange("(g p) c -> p g c", p=128),
            st[:, col0:col0 + g_full, :])
    rem = rows - g_full * 128
    if rem:
        nc.sync.dma_start(dst[r0 + g_full * 128:r0 + rows, 0:width],
                          st[0:rem, col0 + g_full, :])


# ----------------------------------------------------------------------------
# entry point
# ----------------------------------------------------------------------------

_CACHE = {}


def kernel(x, edge_index, edge_attr, W1, att_src1, att_dst1, b1,
           W2, att_src2, att_dst2, b2, ff1_w, ff1_b, ff2_w, ff2_b):
    x = np.asarray(x, np.float32)
    edge_index = np.asarray(edge_index)
    args = [np.asarray(a, np.float32) for a in
            (W1, att_src1, att_dst1, b1, W2, att_src2, att_dst2, b2,
             ff1_w, ff1_b, ff2_w, ff2_b)]
    in_maps, sched, dims = _prep(x, edge_index, *args)
    key = (dims["N"], dims["IN"], tuple(sched.n_lo), tuple(sched.n_hi))
    if key not in _CACHE:
        _CACHE[key] = _build(sched, dims)
    nc = _CACHE[key]
    salt = np.zeros((1, 4), np.float32)
    for m in in_maps:
        m[nc._salt_name] = salt
    res = run_bass_kernel_spmd(nc, in_maps, list(range(NCORES))).results
    out = np.concatenate([res[k]["out"] for k in range(NCORES)], axis=0)
    return out.astype(np.float32)

